# revision 1
# baseline (speedup 1.0000x reference)
"""Trainium2 Bass kernel for fused segment-mean + linear projection.

Reference computation (for x[N,15], sorted batch[N] in [0,G), W[5,15], b[5]):
    sums[g]  = segment_sum(x, batch)          # [G, 15]
    counts[g]= segment_sum(1, batch)          # [G]
    mean     = sums / max(counts, 1)
    out      = where(counts > 0, mean @ W.T + b, 0)   # [G, 5]

Strategy (8 NeuronCores, data parallel over contiguous graph-id ranges):
  Host (index-only preprocessing, no arithmetic on x beyond repacking):
    - each core owns G/8 consecutive graphs; its nodes are repacked into
      "windows" of GPW=32 graphs x 8192 node-slots (4 chunks of 128
      16-node blocks), each graph zero-padded to a 16-node multiple so
      every block belongs to exactly one graph. Graphs that do not fit
      their window spill whole into a small per-stripe overflow stream.
    - all program shapes / the matmul schedule are data-independent, so
      one SPMD program serves all 8 cores; per-core data differs only in
      the input tables (packed x, block->graph assignment, 1/count, ...).
  Device (per core):
    - DMA the packed x stream block-interleaved (block t -> partition
      t%128, chunk t//128), strided DVE tensor_reduce -> per-block sums
      B[128, NCHUNK*15].
    - PE matmuls with on-device-built one-hot matrices (iota + is_equal
      against a tiny host table) scatter-add the 128 block-sums of each
      chunk into per-quadrant PSUM accumulators ([32 graphs, nstripe*15],
      one bank each; disjoint column writes, single start=True opener).
    - fused mean (multiply by 1/count), then a small DVE projection
      (mean @ W.T + b, empty-graph masking) -> out [G/8, 5].
  Host: concatenate the 8 core outputs.
"""

import sys

for _p in ("/opt/trn_rl_repo",):
    if _p not in sys.path:
        sys.path.insert(0, _p)

import numpy as np
from contextlib import ExitStack

import concourse.bass as bass
import concourse.bacc as bacc
import concourse.tile as tile
from concourse import mybir
from concourse.bass_utils import run_bass_kernel_spmd

P = 128          # partitions
BLK = 16         # nodes per block
D = 15           # feature dim
O = 5            # output dim
GPW = 32         # graphs per window
CPW = 4          # chunks per window (chunk = 128 blocks = 2048 node slots)
SLOTS_W = CPW * P * BLK  # 4096 node slots per window

F32 = mybir.dt.float32


# ----------------------------------------------------------------------------
# host planner
# ----------------------------------------------------------------------------

class Plan:
    """Per-run packing plan. All *shape* fields are uniform across cores."""

    def __init__(self, batch, n_cores, G, W=None, b=None):
        self.W = (np.zeros((O, D), np.float32) if W is None
                  else np.asarray(W, np.float32))
        self.b = (np.zeros(O, np.float32) if b is None
                  else np.asarray(b, np.float32))
        batch = np.asarray(batch)
        N = batch.shape[0]
        assert G % (n_cores * P) == 0
        self.G = G
        self.n_cores = n_cores
        self.gpc = G // n_cores                  # graphs per core
        self.nwin = self.gpc // GPW              # windows per core
        self.nstripe = self.gpc // P             # psum stripes per core
        self.nchunk = self.nwin * CPW            # main-stream chunks per core
        self.lslots = self.nwin * SLOTS_W        # node slots per core
        assert self.nwin % self.nstripe == 0
        self.wps = self.nwin // self.nstripe     # windows per stripe (8)

        bounds = np.searchsorted(batch, np.arange(G + 1))
        counts = np.diff(bounds).astype(np.int64)
        self.counts = counts
        self.inv = (1.0 / np.maximum(counts, 1.0)).astype(np.float32)
        self.nonempty = (counts > 0).astype(np.float32)

        nblk_g = (counts + BLK - 1) // BLK       # blocks per graph

        # ---- window placement (per core) ----
        # placements[c] = list of (graph, slot_base_in_core) for windowed graphs
        # overflow[c][stripe] = list of graphs spilled to that stripe's stream
        self.placements = []
        self.overflow = []
        max_oslots = 8 * BLK  # overflow slots per stripe (uniform; >= actual max)
        for c in range(n_cores):
            g0 = c * self.gpc
            placed = []
            oflow = [[] for _ in range(self.nstripe)]
            for w in range(self.nwin):
                pos = 0
                for j in range(GPW):
                    g = g0 + w * GPW + j
                    need = int(nblk_g[g]) * BLK
                    if need == 0:
                        continue
                    if pos + need <= SLOTS_W:
                        placed.append((g, w * SLOTS_W + pos))
                        pos += need
                    else:
                        oflow[w // self.wps].append(g)
            for s in range(self.nstripe):
                used = sum(int(nblk_g[g]) * BLK for g in oflow[s])
                max_oslots = max(max_oslots, used)
            self.placements.append(placed)
            self.overflow.append(oflow)

        # overflow blocks per stripe: a single partial chunk (K<128 matmuls)
        # when it fits, whole chunks otherwise
        max_oblk = -(-max_oslots // BLK)
        if max_oblk <= P:
            self.osb = max(8, -(-max_oblk // 8) * 8)
            self.ocps = 1
        else:
            self.osb = P
            self.ocps = -(-max_oblk // P)
        self.osps = self.ocps * self.osb * BLK   # overflow slots per stripe
        self.noch = self.nstripe * self.ocps     # total overflow chunks
        self.bounds = bounds
        self.N = N

    def core_tables(self, c, x):
        """Build per-core input arrays. x is the full [N, D] float32 array."""
        lslots, nchunk = self.lslots, self.nchunk
        g0 = c * self.gpc

        idx = np.full(lslots, -1, dtype=np.int64)
        asg = np.full(lslots // BLK, -1.0, dtype=np.float32)  # local graph per block
        for g, base in self.placements[c]:
            s0, cnt = int(self.bounds[g]), int(self.counts[g])
            idx[base : base + cnt] = np.arange(s0, s0 + cnt)
            nb = (cnt + BLK - 1) // BLK
            asg[base // BLK : base // BLK + nb] = g - g0

        # overflow stream: per stripe a fixed region of osps slots
        oidx = np.full(self.nstripe * self.osps, -1, dtype=np.int64)
        oasg = np.full(self.nstripe * self.osps // BLK, -1.0, dtype=np.float32)
        for s in range(self.nstripe):
            pos = s * self.osps
            for g in self.overflow[c][s]:
                s0, cnt = int(self.bounds[g]), int(self.counts[g])
                nb = (cnt + BLK - 1) // BLK
                assert pos + nb * BLK <= (s + 1) * self.osps, "overflow overrun"
                oidx[pos : pos + cnt] = np.arange(s0, s0 + cnt)
                oasg[pos // BLK : pos // BLK + nb] = (g - g0) - s * P
                pos += nb * BLK

        def pack(idx_arr):
            out = x[np.clip(idx_arr, 0, self.N - 1)]
            out[idx_arr < 0] = 0.0
            return np.ascontiguousarray(out, dtype=np.float32)

        xw = pack(idx)                            # [lslots, D]
        xb = pack(oidx)                           # [nstripe*osps, D]

        # block t -> partition t%128, chunk t//128; window of chunk m = m//CPW
        t = np.arange(lslots // BLK)
        asgJ = np.full((P, nchunk), -1.0, dtype=np.float32)
        win_base = (t // P // CPW) * GPW
        vals = np.where(asg >= 0, asg - win_base, -1.0)
        asgJ[t % P, t // P] = vals

        asgO = np.full((P, self.noch), -1.0, dtype=np.float32)
        asgO[: self.osb, :] = oasg.reshape(self.noch, self.osb).T

        def stripe_pack(v):
            # graph g (local) -> [partition g%128, col g//128]
            return np.ascontiguousarray(
                v[g0 : g0 + self.gpc].reshape(self.nstripe, P).T.astype(np.float32)
            )

        # fold 1/count, W, b and the empty-graph mask into two tables:
        #   winv[p, o, s, f] = W[o, f] * inv[g(p, s)]
        #   bne[p, s, o]     = b[o] * nonempty[g(p, s)]
        inv_ps = stripe_pack(self.inv)                       # [P, nstripe]
        ne_ps = stripe_pack(self.nonempty)                   # [P, nstripe]
        winv = (inv_ps[:, None, :, None] *
                self.W[None, :, None, :]).astype(np.float32)  # [P,O,S,D]
        bne = (ne_ps[:, :, None] * self.b[None, None, :]).astype(np.float32)

        return {
            "xw": xw.reshape(-1),
            "xb": xb.reshape(-1),
            "asgJ": asgJ,
            "asgO": asgO,
            "winv": np.ascontiguousarray(winv.reshape(P, -1)),
            "bne": np.ascontiguousarray(bne.reshape(P, -1)),
        }


# ----------------------------------------------------------------------------
# device program
# ----------------------------------------------------------------------------

def build_program(plan, W, b):
    """Build + compile the SPMD Bass program (one program, 8 cores)."""
    nchunk, noch, nstripe = plan.nchunk, plan.noch, plan.nstripe
    lslots = plan.lslots
    wps = plan.wps

    nc = bacc.Bacc("TRN2", target_bir_lowering=False, debug=False)

    xw = nc.dram_tensor("xw", [lslots * D], F32, kind="ExternalInput")
    xb = nc.dram_tensor("xb", [nstripe * plan.osps * D], F32, kind="ExternalInput")
    asgJ = nc.dram_tensor("asgJ", [P, nchunk], F32, kind="ExternalInput")
    asgO = nc.dram_tensor("asgO", [P, noch], F32, kind="ExternalInput")
    winv_t = nc.dram_tensor("winv", [P, O * nstripe * D], F32, kind="ExternalInput")
    bne_t = nc.dram_tensor("bne", [P, nstripe * O], F32, kind="ExternalInput")
    out_t = nc.dram_tensor("out", [plan.gpc * O], F32, kind="ExternalOutput")

    CB = 240  # elements per block (BLK * D)
    # x tiles: chunks per DMA tile. Tapered: small first tile so DVE starts
    # early, small last tiles so the post-DMA tail (reduce+route+proj) is short.
    KCS = []
    rem = nchunk
    KCS.append(min(8, rem)); rem -= KCS[-1]
    while rem - 56 >= 32:
        KCS.append(32); rem -= 32
    while rem > 16:
        KCS.append(min(16, rem)); rem -= KCS[-1]
    while rem > 0:
        KCS.append(min(8, rem)); rem -= KCS[-1]
    assert sum(KCS) == nchunk

    with tile.TileContext(nc) as tc, ExitStack() as ctx:
        consts = ctx.enter_context(tc.tile_pool(name="consts", bufs=1))
        xpool = ctx.enter_context(tc.tile_pool(name="xpool", bufs=3))
        bpool = ctx.enter_context(tc.tile_pool(name="bpool", bufs=1))
        ppool = ctx.enter_context(tc.tile_pool(name="ppool", bufs=1, space="PSUM"))

        def ap_of(handle, offset, pattern):
            return bass.AP(tensor=handle.ap().tensor, offset=offset, ap=pattern)

        # ---- constant tables (ACT HWDGE ring; keeps SP ring free for x tiles) ----
        asgJ_sb = consts.tile([P, nchunk], F32)
        nc.scalar.dma_start(out=asgJ_sb[:], in_=asgJ.ap())
        asgO_sb = consts.tile([P, noch], F32)
        nc.scalar.dma_start(out=asgO_sb[:], in_=asgO.ap())
        winv_sb = consts.tile([P, O * nstripe * D], F32)
        bne_sb = consts.tile([P, nstripe * O], F32)

        # ---- iota rows for one-hot construction ----
        iota_w = consts.tile([P, GPW], F32)
        nc.gpsimd.iota(
            iota_w[:],
            pattern=[[1, GPW]],
            base=0,
            channel_multiplier=0,
            allow_small_or_imprecise_dtypes=True,
        )
        iota_o = consts.tile([P, P], F32)
        nc.gpsimd.iota(
            iota_o[:],
            pattern=[[1, P]],
            base=0,
            channel_multiplier=0,
            allow_small_or_imprecise_dtypes=True,
        )
        # identity selection matrix for the quadrant recombine:
        # i4_sb[k, q*P + m] = 1.0 iff m == q*GPW + k
        i4_sb = consts.tile([GPW, (P // GPW) * P], F32)
        nc.gpsimd.memset(i4_sb[:], 0.0)
        nc.gpsimd.affine_select(
            out=i4_sb[:],
            in_=i4_sb[:],
            compare_op=mybir.AluOpType.not_equal,
            fill=1.0,
            base=0,
            channel_multiplier=-1,
            pattern=[[-GPW, P // GPW], [1, P]],
        )
        # one-hot arenas:
        #   onehot[p, m*GPW + w] = (asgJ[p, m] == w)   built per-stripe, DVE,
        #     interleaved into the reduce stream (see emit_oh below)
        #   oneO[p, ch*P + w]   = (asgO[p, ch] == w)   one small DVE op
        onehot = bpool.tile([P, nchunk * GPW], F32)
        oneO = bpool.tile([P, noch * P], F32)
        cps = nchunk // nstripe  # main-stream chunks per stripe

        def emit_oh(s):
            return nc.vector.tensor_tensor(
                out=bass.AP(
                    tensor=onehot.tensor, offset=onehot.offset + s * cps * GPW,
                    ap=[onehot.ap[0], [GPW, cps], [1, GPW]],
                ),
                in0=bass.AP(
                    tensor=asgJ_sb.tensor, offset=asgJ_sb.offset + s * cps,
                    ap=[asgJ_sb.ap[0], [1, cps], [0, GPW]],
                ),
                in1=bass.AP(
                    tensor=iota_w.tensor, offset=iota_w.offset,
                    ap=[iota_w.ap[0], [0, cps], [1, GPW]],
                ),
                op=mybir.AluOpType.is_equal,
            )

        def emit_oo():
            return nc.vector.tensor_tensor(
                out=bass.AP(
                    tensor=oneO.tensor, offset=oneO.offset,
                    ap=[oneO.ap[0], [P, noch], [1, P]],
                ),
                in0=bass.AP(
                    tensor=asgO_sb.tensor, offset=asgO_sb.offset,
                    ap=[asgO_sb.ap[0], [1, noch], [0, P]],
                ),
                in1=bass.AP(
                    tensor=iota_o.tensor, offset=iota_o.offset,
                    ap=[iota_o.ap[0], [0, noch], [1, P]],
                ),
                op=mybir.AluOpType.is_equal,
            )

        # ---- overflow stream: load (reduce emitted after first x reduces) ----
        osb = plan.osb
        xb_sb = bpool.tile([P, noch * CB], F32)
        nc.scalar.dma_start(
            out=xb_sb[:osb, :],
            in_=ap_of(xb, 0, [[CB, osb], [CB * osb, noch], [1, CB]]),
        )
        Bo = bpool.tile([P, noch * D], F32)

        def emit_bo_reduce():
            return nc.vector.tensor_reduce(
                out=bass.AP(
                    tensor=Bo.tensor, offset=Bo.offset,
                    ap=[[Bo.ap[0][0], osb], [D, noch], [1, D]],
                ),
                in_=bass.AP(
                    tensor=xb_sb.tensor, offset=xb_sb.offset,
                    ap=[[xb_sb.ap[0][0], osb], [CB, noch], [1, D], [D, BLK]],
                ),
                axis=mybir.AxisListType.X,
                op=mybir.AluOpType.add,
            )

        # ---- main stream: tapered tiles -> block sums B ----
        B = bpool.tile([P, nchunk * D], F32)
        KCMAX = max(KCS)
        c0 = 0
        oh_next = 0
        reds = []
        for ti, KC in enumerate(KCS):
            xt = xpool.tile([P, KCMAX * CB], F32, tag="xt", name="xt")
            nc.sync.dma_start(
                out=xt[:, : KC * CB],
                in_=ap_of(
                    xw, c0 * P * CB,
                    [[CB, P], [CB * P, KC], [1, CB]],
                ),
            )
            red = nc.vector.tensor_reduce(
                out=bass.AP(
                    tensor=B.tensor, offset=B.offset + c0 * D,
                    ap=[B.ap[0], [D, KC], [1, D]],
                ),
                in_=bass.AP(
                    tensor=xt.tensor, offset=xt.offset,
                    ap=[xt.ap[0], [CB, KC], [1, D], [D, BLK]],
                ),
                axis=mybir.AxisListType.X,
                op=mybir.AluOpType.add,
            )
            c0 += KC
            reds.append(red)
            # Order the small DVE ops (one-hot builds, overflow reduce) AFTER
            # this tile's reduce so the scheduler cannot hoist them ahead of
            # the reduce pipeline (that would starve the x-DMA slot rotation),
            # and pack them into the EARLY tiles so the late tiles' reduces
            # run back-to-back (short post-DMA tail).
            if ti == min(2, len(KCS) - 1):
                tile.add_dep_helper(emit_bo_reduce().ins, red.ins, sync=False,
                                    reason="keep Bo reduce behind tile reduces")
            if ti == min(3, len(KCS) - 1):
                tile.add_dep_helper(emit_oo().ins, red.ins, sync=False,
                                    reason="keep oneO build behind tile reduces")
            quota = 2 if ti == 0 else 4 * ti + 2
            while oh_next < nstripe and (
                oh_next < quota or ti == len(KCS) - 1
            ):
                tile.add_dep_helper(emit_oh(oh_next).ins, red.ins, sync=False,
                                    reason="keep onehot build behind tile reduces")
                oh_next += 1

        # winv/bne loads: only the projection needs them, so keep the
        # 0.66 MB transfer out of the x-stream DMA window.
        wdma = nc.gpsimd.dma_start(out=winv_sb[:], in_=winv_t.ap())
        bdma = nc.gpsimd.dma_start(out=bne_sb[:], in_=bne_t.ap())
        if len(reds) >= 3:
            tile.add_dep_helper(wdma.ins, reds[-3].ins, sync=False,
                                reason="winv load off the x-stream window")
            tile.add_dep_helper(bdma.ins, reds[-3].ins, sync=False,
                                reason="bne load off the x-stream window")

        # ---- routing matmuls ----
        # Each 32-graph quadrant accumulates ALL stripes into one PSUM tile
        # [GPW, nstripe*D] (960B — fits one bank; stripe s owns columns
        # s*D..(s+1)*D). One start=True opener per quadrant clears the bank's
        # has_written bits; every other matmul accumulates-or-overwrites its
        # disjoint region, which is exact for disjoint column writes.
        nquad = P // GPW
        psums = [ppool.tile([GPW, nstripe * D], F32, name=f"ps{q}")
                 for q in range(nquad)]
        openers = [None] * nquad
        for s in range(nstripe):
            for q in range(nquad):
                psum = psums[q]
                mms = []
                for j in range(CPW):
                    m = (s * wps + q) * CPW + j
                    mms.append(nc.tensor.matmul(
                        out=psum[:, s * D : (s + 1) * D],
                        lhsT=onehot[:, m * GPW : (m + 1) * GPW],
                        rhs=B[:, m * D : (m + 1) * D],
                        start=(s == 0 and j == 0),
                        stop=(s == nstripe - 1 and j == CPW - 1),
                        tile_position=(0, 0),
                        skip_group_check=True,
                    ))
                for oc in range(plan.ocps):
                    ch = s * plan.ocps + oc
                    mms.append(nc.tensor.matmul(
                        out=psum[:, s * D : (s + 1) * D],
                        lhsT=oneO[:osb, ch * P + q * GPW : ch * P + (q + 1) * GPW],
                        rhs=Bo[:osb, ch * D : (ch + 1) * D],
                        start=False,
                        stop=False,
                        tile_position=(0, 0),
                        skip_group_check=True,
                    ))
                if s == 0:
                    openers[q] = mms[0]
                    mms = mms[1:]
                # the opener's bank-wide has_written clear must run first
                for mm in mms:
                    tile.add_dep_helper(mm.ins, openers[q].ins, sync=False,
                                        reason="psum opener first")

        # flush each quadrant once (ACT), then recombine on PE via a constant
        # identity selection matmul into a single [128, nstripe*D] PSUM tile
        sums_q = [bpool.tile([GPW, nstripe * D], F32, name=f"sumsq{q}")
                  for q in range(nquad)]
        for q in range(nquad):
            eng = nc.scalar.copy if q % 2 == 0 else nc.vector.tensor_copy
            eng(out=sums_q[q][:, :], in_=psums[q][:, :])
        psum_all = ppool.tile([P, nstripe * D], F32)
        for q in range(nquad):
            nc.tensor.matmul(
                out=psum_all[:, :],
                lhsT=i4_sb[:, q * P : (q + 1) * P],
                rhs=sums_q[q][:, :],
                start=(q == 0),
                stop=(q == nquad - 1),
                tile_position=(0, 0),
                skip_group_check=True,
            )


        # projection straight from PSUM, in two stripe-halves so the first
        # half's output DMA overlaps the second half's DVE work:
        #   tmp[p,o,s,f] = psum_all[p,s,f] * winv[p,o,s,f]
        #   proj[p,s*O+o] = sum_f tmp ;  out = proj + bne
        proj = bpool.tile([P, nstripe * O], F32)
        tmp = bpool.tile([P, O * nstripe * D], F32)
        outv = bpool.tile([P, nstripe * O], F32)
        sh = max(1, nstripe // 2)
        s0 = 0
        while s0 < nstripe:
            sn = min(sh, nstripe - s0)
            nc.vector.tensor_tensor(
                out=bass.AP(
                    tensor=tmp.tensor, offset=tmp.offset + s0 * D,
                    ap=[tmp.ap[0], [nstripe * D, O], [D, sn], [1, D]],
                ),
                in0=bass.AP(
                    tensor=psum_all.tensor, offset=psum_all.offset + s0 * D,
                    ap=[psum_all.ap[0], [0, O], [D, sn], [1, D]],
                ),
                in1=bass.AP(
                    tensor=winv_sb.tensor, offset=winv_sb.offset + s0 * D,
                    ap=[winv_sb.ap[0], [nstripe * D, O], [D, sn], [1, D]],
                ),
                op=mybir.AluOpType.mult,
            )
            nc.vector.tensor_reduce(
                out=bass.AP(
                    tensor=proj.tensor, offset=proj.offset + s0 * O,
                    ap=[proj.ap[0], [1, O], [O, sn], [1, 1]],
                ),
                in_=bass.AP(
                    tensor=tmp.tensor, offset=tmp.offset + s0 * D,
                    ap=[tmp.ap[0], [nstripe * D, O], [D, sn], [1, D]],
                ),
                axis=mybir.AxisListType.X,
                op=mybir.AluOpType.add,
            )
            # out = proj + b*nonempty  (empty graphs have exact 0 in proj)
            nc.vector.tensor_tensor(
                out=outv[:, s0 * O : (s0 + sn) * O],
                in0=proj[:, s0 * O : (s0 + sn) * O],
                in1=bne_sb[:, s0 * O : (s0 + sn) * O],
                op=mybir.AluOpType.add,
            )
            nc.sync.dma_start(
                out=ap_of(out_t, s0 * P * O, [[O, P], [P * O, sn], [1, O]]),
                in_=outv[:, s0 * O : (s0 + sn) * O],
            )
            s0 += sn

    nc.compile()
    return nc


# ----------------------------------------------------------------------------
# entry point
# ----------------------------------------------------------------------------

_CACHE = {}
_LAST_RESULTS = None


def kernel(x, batch, W, b):
    global _LAST_RESULTS
    x = np.asarray(x, dtype=np.float32)
    batch = np.asarray(batch)
    W = np.asarray(W, dtype=np.float32)
    b = np.asarray(b, dtype=np.float32)

    n_cores = 8
    G = 16384
    plan = Plan(batch, n_cores, G, W, b)

    key = (plan.lslots, plan.nchunk, plan.noch, plan.osps)
    if key not in _CACHE:
        _CACHE[key] = build_program(plan, W, b)
    nc = _CACHE[key]

    in_maps = [plan.core_tables(c, x) for c in range(n_cores)]

    def _run():
        return run_bass_kernel_spmd(nc, in_maps, core_ids=list(range(n_cores)))

    try:
        res = _run()
    except ModuleNotFoundError:
        # BASS_TRACE was set but this container lacks the axon NTFF profiling
        # hook (antenv.axon_hooks) — retry with tracing disabled.
        import os
        os.environ["BASS_NEVER_TRACE"] = "1"
        res = _run()
    except Exception as e:  # transient device/terminal failure -> one retry
        if not any(k in str(e) for k in ("UNAVAILABLE", "UNRECOVERABLE")):
            raise
        import time as _time
        _time.sleep(10.0)
        res = _run()
    _LAST_RESULTS = res
    out = np.concatenate(
        [res.results[c]["out"].reshape(plan.gpc, O) for c in range(n_cores)], axis=0
    )
    return out.astype(np.float32)


if __name__ == "__main__":
    # tiny smoke test of the planner only
    rng = np.random.default_rng(0)
    N, G = 400_000, 16384
    batch = np.sort(rng.integers(0, G, N))
    x = rng.standard_normal((N, D), dtype=np.float32)
    plan = Plan(batch, 8, G)
    print("lslots", plan.lslots, "nchunk", plan.nchunk, "osps", plan.osps)
    t = plan.core_tables(0, x)
    for k, v in t.items():
        print(k, v.shape, v.dtype)



# revision 19
# speedup vs baseline: 1.8029x; 1.8029x over previous
"""Trainium2 Bass kernel for fused segment-mean + linear projection.

Reference computation (for x[N,15], sorted batch[N] in [0,G), W[5,15], b[5]):
    sums[g]  = segment_sum(x, batch)          # [G, 15]
    counts[g]= segment_sum(1, batch)          # [G]
    mean     = sums / max(counts, 1)
    out      = where(counts > 0, mean @ W.T + b, 0)   # [G, 5]

Strategy (8 NeuronCores, data parallel over contiguous graph-id ranges):
  Host (index-only preprocessing; x is repacked and cast to bf16):
    - each core owns G/8 consecutive graphs; its nodes are repacked into
      "windows" of GPW=32 graphs x 8192 node-slots (4 chunks of 128
      16-node blocks), each graph zero-padded to a 16-node multiple so
      every block belongs to exactly one graph. Graphs that do not fit
      their window spill whole into a small per-stripe overflow stream.
    - the packed stream is bf16, PARTITION-MAJOR (each SBUF partition's
      whole chunk range is contiguous in DRAM -> large DMA descriptors),
      and each 16-node block is stored feature-major [15 feats x 16
      nodes] so the on-device block-sum tree has stride-1 innermost
      access (DVE 2x mode).
    - all program shapes / the matmul schedule are data-independent, so
      one SPMD program serves all 8 cores; per-core data differs only in
      the input tables.
  Device (per core):
    - DMA bf16 x tiles; 4-stage DVE tensor_tensor halving tree
      (16->8->4->2->1 nodes) -> per-block sums B[128, NCHUNK*15] (bf16).
    - PE bf16 matmuls with on-device-built one-hot matrices scatter-add
      the block sums into per-quadrant PSUM accumulators (f32, exact).
    - flush each quadrant through a DVE multiply by 1/count (per-graph)
      -> bf16 means; PE identity matmul recombines quadrants to 128
      rows; DVE multiply by replicated W + reduce + bias -> out [G/8,5].
    - work is processed in stripe-quarters so the flush/recombine/
      projection pipeline overlaps the x stream instead of trailing it.
  Host: concatenate the 8 core outputs.
"""

import sys

for _p in ("/opt/trn_rl_repo",):
    if _p not in sys.path:
        sys.path.insert(0, _p)

import numpy as np
import ml_dtypes
from contextlib import ExitStack

import concourse.bass as bass
import concourse.bacc as bacc
import concourse.tile as tile
from concourse import mybir
from concourse.bass_utils import run_bass_kernel_spmd

P = 128          # partitions
BLK = 16         # nodes per block
D = 15           # feature dim
O = 5            # output dim
GPW = 32         # graphs per window
CPW = 4          # chunks per window (chunk = 128 blocks = 2048 node slots)
SLOTS_W = CPW * P * BLK  # node slots per window

F32 = mybir.dt.float32
BF16 = mybir.dt.bfloat16
BNP = ml_dtypes.bfloat16


# ----------------------------------------------------------------------------
# host planner
# ----------------------------------------------------------------------------

class Plan:
    """Per-run packing plan. All *shape* fields are uniform across cores."""

    def __init__(self, batch, n_cores, G, W=None, b=None):
        self.W = (np.zeros((O, D), np.float32) if W is None
                  else np.asarray(W, np.float32))
        self.b = (np.zeros(O, np.float32) if b is None
                  else np.asarray(b, np.float32))
        batch = np.asarray(batch)
        N = batch.shape[0]
        assert G % (n_cores * P) == 0
        self.G = G
        self.n_cores = n_cores
        self.gpc = G // n_cores                  # graphs per core
        self.nwin = self.gpc // GPW              # windows per core
        self.nstripe = self.gpc // P             # psum stripes per core
        self.nchunk = self.nwin * CPW            # main-stream chunks per core
        self.lslots = self.nwin * SLOTS_W        # node slots per core
        assert self.nwin % self.nstripe == 0
        self.wps = self.nwin // self.nstripe     # windows per stripe
        self.cps = self.nchunk // self.nstripe   # chunks per stripe

        bounds = np.searchsorted(batch, np.arange(G + 1))
        counts = np.diff(bounds).astype(np.int64)
        self.counts = counts
        self.inv = (1.0 / np.maximum(counts, 1.0)).astype(np.float32)
        self.nonempty = (counts > 0).astype(np.float32)

        nblk_g = (counts + BLK - 1) // BLK       # blocks per graph

        # ---- window placement (per core) ----
        self.placements = []
        self.overflow = []
        max_oslots = 8 * BLK
        for c in range(n_cores):
            g0 = c * self.gpc
            placed = []
            oflow = [[] for _ in range(self.nstripe)]
            for w in range(self.nwin):
                pos = 0
                for j in range(GPW):
                    g = g0 + w * GPW + j
                    need = int(nblk_g[g]) * BLK
                    if need == 0:
                        continue
                    if pos + need <= SLOTS_W:
                        placed.append((g, w * SLOTS_W + pos))
                        pos += need
                    else:
                        oflow[w // self.wps].append(g)
            for s in range(self.nstripe):
                used = sum(int(nblk_g[g]) * BLK for g in oflow[s])
                max_oslots = max(max_oslots, used)
            self.placements.append(placed)
            self.overflow.append(oflow)

        # overflow blocks per stripe: a single partial chunk (K<128 matmuls)
        # when it fits, whole chunks otherwise
        max_oblk = -(-max_oslots // BLK)
        if max_oblk <= P:
            self.osb = max(8, -(-max_oblk // 8) * 8)
            self.ocps = 1
        else:
            self.osb = P
            self.ocps = -(-max_oblk // P)
        self.osps = self.ocps * self.osb * BLK   # overflow slots per stripe
        self.noch = self.nstripe * self.ocps     # total overflow chunks
        self.bounds = bounds
        self.N = N

    def core_tables(self, c, x):
        """Build per-core input arrays. x is the full [N, D] float32 array."""
        lslots, nchunk = self.lslots, self.nchunk
        g0 = c * self.gpc

        idx = np.full(lslots, -1, dtype=np.int64)
        asg = np.full(lslots // BLK, -1.0, dtype=np.float32)  # local graph/blk
        for g, base in self.placements[c]:
            s0, cnt = int(self.bounds[g]), int(self.counts[g])
            idx[base : base + cnt] = np.arange(s0, s0 + cnt)
            nb = (cnt + BLK - 1) // BLK
            asg[base // BLK : base // BLK + nb] = g - g0

        # overflow stream: per stripe a fixed region of osps slots
        oidx = np.full(self.nstripe * self.osps, -1, dtype=np.int64)
        oasg = np.full(self.nstripe * self.osps // BLK, -1.0, dtype=np.float32)
        for s in range(self.nstripe):
            pos = s * self.osps
            for g in self.overflow[c][s]:
                s0, cnt = int(self.bounds[g]), int(self.counts[g])
                nb = (cnt + BLK - 1) // BLK
                assert pos + nb * BLK <= (s + 1) * self.osps, "overflow overrun"
                oidx[pos : pos + cnt] = np.arange(s0, s0 + cnt)
                oasg[pos // BLK : pos // BLK + nb] = (g - g0) - s * P
                pos += nb * BLK

        def pack(idx_arr, nch, parts):
            # gather, then block t -> (partition t % parts, chunk t // parts),
            # partition-major layout, blocks in natural node-major order
            # (so every halving-tree stage is stride-1 innermost -> DVE 2x)
            out = x[np.clip(idx_arr, 0, self.N - 1)]
            out[idx_arr < 0] = 0.0
            out = out.reshape(nch, parts, BLK * D).transpose(1, 0, 2)
            return np.ascontiguousarray(out.reshape(parts, nch * BLK * D)
                                        .astype(BNP))

        xw = pack(idx, nchunk, P)                 # [P, nchunk*CB] bf16
        xb = pack(oidx, self.noch, self.osb)      # [osb, noch*CB] bf16

        # block t -> partition t%128, chunk t//128; window of chunk m = m//CPW
        t = np.arange(lslots // BLK)
        asgJ = np.full((P, nchunk), -1.0, dtype=np.float32)
        win_base = (t // P // CPW) * GPW
        vals = np.where(asg >= 0, asg - win_base, -1.0)
        asgJ[t % P, t // P] = vals

        asgO = np.full((P, self.noch), -1.0, dtype=np.float32)
        asgO[: self.osb, :] = oasg.reshape(self.noch, self.osb).T

        def stripe_pack(v):
            # graph g (local) -> [partition g%128, col g//128]
            return np.ascontiguousarray(
                v[g0 : g0 + self.gpc].reshape(self.nstripe, P).T.astype(np.float32)
            )

        inv_ps = stripe_pack(self.inv)                       # [P, nstripe] f32
        # fused W x 1/count table: winv[p, o, s, f] = W[o, f] * inv[g(p, s)]
        winv = (inv_ps[:, None, :, None] *
                self.W[None, :, None, :]).astype(BNP)        # [P,O,S,D] bf16
        bne = (stripe_pack(self.nonempty)[:, :, None] *
               self.b[None, None, :]).reshape(P, self.nstripe * O)
        # pad bne to 128 f32 cols so the DMA descriptor is >= 512B
        bne_pad = np.zeros((P, 128), np.float32)
        bne_pad[:, : self.nstripe * O] = bne
        # identity selection matrix for the quadrant recombine:
        # i4[k, q*P + m] = 1.0 iff m == q*GPW + k   (bf16, built on host)
        nq = P // GPW
        i4 = np.zeros((GPW, nq * P), BNP)
        for q in range(nq):
            for k in range(GPW):
                i4[k, q * P + q * GPW + k] = 1.0

        return {
            "xw": xw.reshape(-1),
            "xb": xb.reshape(-1),
            "asgJ": asgJ.astype(BNP),
            "asgO": asgO.astype(BNP),
            "winv": np.ascontiguousarray(winv.reshape(P, -1)),
            "bne": bne_pad,
            "i4": i4,
        }


# ----------------------------------------------------------------------------
# device program
# ----------------------------------------------------------------------------

def build_program(plan, W, b):
    """Build + compile the SPMD Bass program (one program, 8 cores)."""
    nchunk, noch, nstripe = plan.nchunk, plan.noch, plan.nstripe
    cps, wps, osb = plan.cps, plan.wps, plan.osb
    CB = BLK * D     # elements per block (240)

    nc = bacc.Bacc("TRN2", target_bir_lowering=False, debug=False)

    xw = nc.dram_tensor("xw", [P * nchunk * CB], BF16, kind="ExternalInput")
    xb = nc.dram_tensor("xb", [osb * noch * CB], BF16, kind="ExternalInput")
    asgJ = nc.dram_tensor("asgJ", [P, nchunk], BF16, kind="ExternalInput")
    asgO = nc.dram_tensor("asgO", [P, noch], BF16, kind="ExternalInput")
    winv_t = nc.dram_tensor("winv", [P, O * nstripe * D], BF16,
                            kind="ExternalInput")
    bne_t = nc.dram_tensor("bne", [P, 128], F32, kind="ExternalInput")
    i4_t = nc.dram_tensor("i4", [GPW, (P // GPW) * P], BF16,
                          kind="ExternalInput")
    out_t = nc.dram_tensor("out", [plan.gpc * O], F32, kind="ExternalOutput")

    # x tiles: chunks per DMA tile. Tapered: small first tiles so DVE starts
    # early and is never starved, small last tiles so the post-DMA tail is
    # short. Stage-1 of the big middle tiles runs on the (otherwise idle)
    # GPSIMD engine to keep the DVE chain under the DMA roofline.
    KCS = []
    rem = nchunk
    for k in (2, 4, 8, 8, 16, 16):
        kc = min(k, rem)
        if kc:
            KCS.append(kc); rem -= kc
    while rem > 16 + 8 + 4 + 2 + 12:
        KCS.append(min(32, rem - 42)); rem -= KCS[-1]
    for k in (16, 12, 8, 4, 2):
        kc = min(k, rem)
        if kc:
            KCS.append(kc); rem -= kc
    assert sum(KCS) == nchunk and rem == 0
    nquad = P // GPW
    # flush/recombine/projection groups of stripes; the trailing 3+1 split
    # keeps the very last group (one stripe) tiny so the tail is short.
    GROUPS = []
    s = 0
    while s + 4 < nstripe:
        GROUPS.append((s, 4)); s += 4
    if nstripe - s > 1:
        GROUPS.append((s, nstripe - s - 1)); s = nstripe - 1
    GROUPS.append((s, 1))

    with tile.TileContext(nc) as tc, ExitStack() as ctx:
        consts = ctx.enter_context(tc.tile_pool(name="consts", bufs=1))
        xpool = ctx.enter_context(tc.tile_pool(name="xpool", bufs=5))
        spool = ctx.enter_context(tc.tile_pool(name="spool", bufs=2))
        bpool = ctx.enter_context(tc.tile_pool(name="bpool", bufs=1))
        ppool = ctx.enter_context(tc.tile_pool(name="ppool", bufs=2, space="PSUM"))

        def ap_of(handle, offset, pattern):
            return bass.AP(tensor=handle.ap().tensor, offset=offset, ap=pattern)

        def tap(t, offset, pattern):
            return bass.AP(tensor=t.tensor, offset=t.offset + offset, ap=pattern)

        # ---- constant tables (ACT HWDGE ring; keeps SP ring free for x) ----
        # Only asgJ is needed immediately (first one-hot builds); the rest is
        # pinned behind the early x tiles so the x stream owns the DMA bus.
        asgJ_sb = consts.tile([P, nchunk], BF16)
        asgO_sb = consts.tile([P, noch], BF16)
        i4_sb = consts.tile([GPW, nquad * P], BF16)
        winv_sb = consts.tile([P, O * nstripe * D], BF16)
        bne_sb = consts.tile([P, 128], F32)

        # ---- iota tables for one-hot construction (w-major layouts) ----
        # iota_rep[p, w*cps + m] = w   for the main-stream one-hot
        iota_rep = consts.tile([P, GPW * cps], BF16)
        nc.gpsimd.iota(
            iota_rep[:],
            pattern=[[1, GPW], [0, cps]],
            base=0,
            channel_multiplier=0,
            allow_small_or_imprecise_dtypes=True,
        )
        # iota_rep_o[p, w*noch + ch] = w  for the overflow one-hot
        iota_rep_o = consts.tile([P, P * noch], BF16)
        nc.gpsimd.iota(
            iota_rep_o[:],
            pattern=[[1, P], [0, noch]],
            base=0,
            channel_multiplier=0,
            allow_small_or_imprecise_dtypes=True,
        )

        # one-hot arenas (w-major within each stripe so the is_equal build
        # has stride-1 innermost on every operand -> DVE 2x mode):
        #   onehot[p, s*cps*GPW + w*cps + ml] = (asgJ[p, s*cps+ml] == w)
        #   oneO[p, w*noch + ch]              = (asgO[p, ch] == w)
        onehot = bpool.tile([P, nchunk * GPW], BF16)
        oneO = bpool.tile([P, P * noch], BF16)

        def emit_oh(s0, ns):
            # one op builds the one-hot arena for ns stripes (w-major per
            # stripe; every operand has stride-1 innermost -> DVE 2x mode)
            return nc.vector.tensor_tensor(
                out=tap(onehot, s0 * cps * GPW,
                        [onehot.ap[0], [cps * GPW, ns], [cps, GPW], [1, cps]]),
                in0=tap(asgJ_sb, s0 * cps,
                        [asgJ_sb.ap[0], [cps, ns], [0, GPW], [1, cps]]),
                in1=tap(iota_rep, 0,
                        [iota_rep.ap[0], [0, ns], [cps, GPW], [1, cps]]),
                op=mybir.AluOpType.is_equal,
            )

        def emit_oo():
            return nc.vector.tensor_tensor(
                out=tap(oneO, 0,
                        [[oneO.ap[0][0], osb], [noch, P], [1, noch]]),
                in0=tap(asgO_sb, 0,
                        [[asgO_sb.ap[0][0], osb], [0, P], [1, noch]]),
                in1=tap(iota_rep_o, 0,
                        [[iota_rep_o.ap[0][0], osb], [noch, P], [1, noch]]),
                op=mybir.AluOpType.is_equal,
            )

        # ---- block-sum halving tree (DVE tensor_tensor, 2x mode) ----
        # src layout per chunk-column: [D, W] feature-major, W nodes.
        def emit_tree(src, soff, dst, doff, kc, parts, tag):
            # node-major halving tree: every stage adds two contiguous
            # 15-element runs (stride-1 innermost on all operands -> 2x)
            cur, co, w = src, soff, BLK
            last = None
            while w > 1:
                h = w // 2
                if h > 1:
                    nxt = spool.tile([P, kc * D * h], BF16,
                                     tag=f"{tag}{h}", name=f"{tag}{h}")
                    no = 0
                else:
                    nxt, no = dst, doff
                cp = [cur.ap[0][0], parts]
                last = nc.vector.tensor_tensor(
                    out=tap(nxt, no, [[nxt.ap[0][0], parts],
                                      [D * h, kc], [D, h], [1, D]]),
                    in0=tap(cur, co, [cp, [D * w, kc], [D, h], [1, D]]),
                    in1=tap(cur, co + h * D,
                            [cp, [D * w, kc], [D, h], [1, D]]),
                    op=mybir.AluOpType.add,
                )
                cur, co, w = nxt, no, h
            return last

        # ---- overflow stream ----
        xb_sb = bpool.tile([P, noch * CB], BF16)
        Bo = bpool.tile([P, noch * D], BF16)

        # ---- main stream: tapered tiles -> block sums B ----
        B = bpool.tile([P, nchunk * D], BF16)
        KCMAX = max(KCS)
        c0 = 0
        oh_next = 0
        reds = []
        dmas = []
        for ti, KC in enumerate(KCS):
            xt = xpool.tile([P, KCMAX * CB], BF16, tag="xt", name="xt")
            xdma = nc.sync.dma_start(
                out=xt[:, : KC * CB],
                in_=ap_of(xw, c0 * CB, [[nchunk * CB, P], [1, KC * CB]]),
            )
            dmas.append(xdma)
            red = emit_tree(xt, 0, B, c0 * D, KC, P, "st")
            c0 += KC
            reds.append(red)
            # secondary table loads, pinned behind the early x tiles so the
            # x stream owns the DMA bus at the start
            if ti == 0:
                for dma in (
                    nc.scalar.dma_start(out=asgJ_sb[:], in_=asgJ.ap()),
                    nc.scalar.dma_start(out=asgO_sb[:], in_=asgO.ap()),
                    nc.scalar.dma_start(
                        out=xb_sb[:osb, :],
                        in_=ap_of(xb, 0, [[noch * CB, osb], [1, noch * CB]]),
                    ),
                ):
                    tile.add_dep_helper(dma.ins, dmas[0].ins, sync=False,
                                        reason="table loads behind x tiles")
            if ti == 3:
                for dma in (
                    nc.scalar.dma_start(out=i4_sb[:], in_=i4_t.ap()),
                    nc.scalar.dma_start(out=winv_sb[:], in_=winv_t.ap()),
                    nc.scalar.dma_start(out=bne_sb[:], in_=bne_t.ap()),
                ):
                    tile.add_dep_helper(dma.ins, dmas[3].ins, sync=False,
                                        reason="table loads behind x tiles")
            # Pack the small side-jobs behind the early tiles' tree work so
            # the scheduler cannot hoist them ahead of the x pipeline.
            if ti == 0:
                oh = emit_oh(0, nstripe // 2)
                tile.add_dep_helper(oh.ins, red.ins, sync=False,
                                    reason="onehot build in early DVE idle")
            if ti == min(2, len(KCS) - 1):
                bo = emit_tree(xb_sb, 0, Bo, 0, noch, osb, "ob")
                tile.add_dep_helper(bo.ins, red.ins, sync=False,
                                    reason="Bo tree in early DVE idle")
            if ti == min(3, len(KCS) - 1):
                tile.add_dep_helper(emit_oo().ins, red.ins, sync=False,
                                    reason="oneO build in early DVE idle")
            if ti == min(7, len(KCS) - 1):
                oh = emit_oh(nstripe // 2, nstripe - nstripe // 2)
                tile.add_dep_helper(oh.ins, red.ins, sync=False,
                                    reason="onehot half 2 in mid-stream idle")

        # ---- routing matmuls, grouped by stripe-group ----
        # Each group accumulates its stripes into ONE combined quadrant PSUM
        # tile [GPW, nquad*qs*D] (single bank; quadrant q owns columns
        # q*qs*D..): one start=True opener per group clears the bank, all
        # other matmuls accumulate disjoint regions. Per-group start/stop
        # lets the flush/recombine/projection pipeline run mid-stream
        # instead of trailing the whole x stream.
        QSMAX = max(qs for _, qs in GROUPS)
        sums_g = [bpool.tile([GPW, nquad * QSMAX * D], BF16, name=f"sums{g}")
                  for g in range(len(GROUPS))]
        tmp = bpool.tile([P, len(GROUPS) * O * QSMAX * D], F32)
        proj = bpool.tile([P, nstripe * O], F32)
        outv = bpool.tile([P, nstripe * O], F32)

        for g, (s0, qs) in enumerate(GROUPS):
            qsum = ppool.tile([GPW, nquad * QSMAX * D], F32,
                              tag="qsum", name=f"qsum{g}")
            opener = None
            last_mm = None
            for s in range(s0, s0 + qs):
                for q in range(nquad):
                    col = (q * qs + (s - s0)) * D
                    mms = []
                    for j in range(CPW):
                        m = (s * wps + q) * CPW + j
                        ml = m - s * cps
                        mms.append(nc.tensor.matmul(
                            out=qsum[:, col : col + D],
                            lhsT=tap(onehot, s * cps * GPW + ml,
                                     [onehot.ap[0], [cps, GPW]]),
                            rhs=B[:, m * D : (m + 1) * D],
                            start=(opener is None and not mms),
                            stop=False,
                            tile_position=(0, 0),
                            skip_group_check=True,
                        ))
                    glast = (s == s0 + qs - 1) and (q == nquad - 1)
                    for oc in range(plan.ocps):
                        ch = s * plan.ocps + oc
                        mms.append(nc.tensor.matmul(
                            out=qsum[:, col : col + D],
                            lhsT=tap(oneO, ch + q * GPW * noch,
                                     [[oneO.ap[0][0], osb], [noch, GPW]]),
                            rhs=Bo[:osb, ch * D : (ch + 1) * D],
                            start=False,
                            stop=(glast and oc == plan.ocps - 1),
                            tile_position=(0, 0),
                            skip_group_check=True,
                        ))
                    if opener is None:
                        opener = mms[0]
                        mms = mms[1:]
                    for mm in mms:
                        tile.add_dep_helper(mm.ins, opener.ins, sync=False,
                                            reason="psum opener first")
                    last_mm = mms[-1] if mms else opener

            # flush: one ACT copy, PSUM f32 -> SBUF bf16 (1/count is in winv)
            nc.scalar.copy(
                out=sums_g[g][:, : nquad * qs * D],
                in_=qsum[:, : nquad * qs * D],
            )

            # recombine quadrants -> pall [P, qs*D] (PE, bf16 exact)
            pall = ppool.tile([P, QSMAX * D], F32, tag="pall", name=f"pall{g}")
            for q in range(nquad):
                nc.tensor.matmul(
                    out=pall[:, : qs * D],
                    lhsT=i4_sb[:, q * P : (q + 1) * P],
                    rhs=sums_g[g][:, q * qs * D : (q + 1) * qs * D],
                    start=(q == 0),
                    stop=(q == nquad - 1),
                    tile_position=(0, 0),
                    skip_group_check=True,
                )
            # projection: tmp[p,o,s,f] = mean * W[o,f]*inv; reduce f; + bias
            toff = g * O * QSMAX * D
            nc.vector.tensor_tensor(
                out=tap(tmp, toff, [tmp.ap[0], [qs * D, O], [D, qs], [1, D]]),
                in0=tap(pall, 0, [pall.ap[0], [0, O], [D, qs], [1, D]]),
                in1=tap(winv_sb, s0 * D,
                        [winv_sb.ap[0], [nstripe * D, O], [D, qs], [1, D]]),
                op=mybir.AluOpType.mult,
            )
            nc.vector.tensor_reduce(
                out=tap(proj, s0 * O, [proj.ap[0], [1, O], [O, qs]]),
                in_=tap(tmp, toff, [tmp.ap[0], [qs * D, O], [D, qs], [1, D]]),
                axis=mybir.AxisListType.X,
                op=mybir.AluOpType.add,
            )
            nc.vector.tensor_tensor(
                out=outv[:, s0 * O : (s0 + qs) * O],
                in0=proj[:, s0 * O : (s0 + qs) * O],
                in1=bne_sb[:, s0 * O : (s0 + qs) * O],
                op=mybir.AluOpType.add,
            )
            (nc.sync if g % 2 == 0 else nc.scalar).dma_start(
                out=ap_of(out_t, s0 * P * O, [[O, P], [P * O, qs], [1, O]]),
                in_=outv[:, s0 * O : (s0 + qs) * O],
            )

    nc.compile()
    return nc


# ----------------------------------------------------------------------------
# entry point
# ----------------------------------------------------------------------------

_CACHE = {}
_LAST_RESULTS = None


def kernel(x, batch, W, b):
    global _LAST_RESULTS
    x = np.asarray(x, dtype=np.float32)
    batch = np.asarray(batch)
    W = np.asarray(W, dtype=np.float32)
    b = np.asarray(b, dtype=np.float32)

    n_cores = 8
    G = 16384
    plan = Plan(batch, n_cores, G, W, b)

    key = (plan.lslots, plan.nchunk, plan.noch, plan.osps)
    if key not in _CACHE:
        _CACHE[key] = build_program(plan, W, b)
    nc = _CACHE[key]

    in_maps = [plan.core_tables(c, x) for c in range(n_cores)]

    def _run():
        return run_bass_kernel_spmd(nc, in_maps, core_ids=list(range(n_cores)))

    try:
        res = _run()
    except ModuleNotFoundError:
        # BASS_TRACE was set but this container lacks the axon NTFF profiling
        # hook (antenv.axon_hooks) — retry with tracing disabled.
        import os
        os.environ["BASS_NEVER_TRACE"] = "1"
        res = _run()
    except Exception as e:  # transient device/terminal failure -> one retry
        if not any(k in str(e) for k in ("UNAVAILABLE", "UNRECOVERABLE")):
            raise
        import time as _time
        _time.sleep(10.0)
        res = _run()
    _LAST_RESULTS = res
    out = np.concatenate(
        [res.results[c]["out"].reshape(plan.gpc, O) for c in range(n_cores)],
        axis=0,
    )
    return out.astype(np.float32)


if __name__ == "__main__":
    # tiny smoke test of the planner only
    rng = np.random.default_rng(0)
    N, G = 400_000, 16384
    batch = np.sort(rng.integers(0, G, N))
    x = rng.standard_normal((N, D), dtype=np.float32)
    plan = Plan(batch, 8, G)
    print("lslots", plan.lslots, "nchunk", plan.nchunk, "osps", plan.osps)
    t = plan.core_tables(0, x)
    for k, v in t.items():
        print(k, v.shape, v.dtype)


# revision 22
# speedup vs baseline: 1.8081x; 1.0029x over previous
"""Trainium2 Bass kernel for fused segment-mean + linear projection.

Reference computation (for x[N,15], sorted batch[N] in [0,G), W[5,15], b[5]):
    sums[g]  = segment_sum(x, batch)          # [G, 15]
    counts[g]= segment_sum(1, batch)          # [G]
    mean     = sums / max(counts, 1)
    out      = where(counts > 0, mean @ W.T + b, 0)   # [G, 5]

Strategy (8 NeuronCores, data parallel over contiguous graph-id ranges):
  Host (index-only preprocessing; x is repacked and cast to bf16 --
  the 2e-2 correctness budget dwarfs bf16 quantization error, and it
  halves the HBM traffic this memory-bound kernel is limited by):
    - each core owns G/8 consecutive graphs; its nodes are repacked into
      "windows" of GPW=32 graphs x 8192 node-slots (4 chunks of 128
      16-node blocks), each graph zero-padded to a 16-node multiple so
      every block belongs to exactly one graph. Graphs that do not fit
      their window spill whole into a small per-stripe overflow stream.
    - the packed stream is bf16 PARTITION-MAJOR: each SBUF partition's
      chunk range is contiguous in DRAM, so DMA descriptors stay >= 512B
      (full 360 GB/s; no small-descriptor penalty) at half the bytes.
    - all program shapes / the matmul schedule are data-independent, so
      one SPMD program serves all 8 cores; per-core data differs only in
      the input tables.
  Device (per core):
    - DMA bf16 x tiles (tapered sizes); block sums via a 4-stage DVE
      tensor_tensor halving tree (16->8->4->2->1 nodes). Blocks are
      node-major so every stage adds two contiguous 15-element runs:
      all operands are 2-byte stride-1 -> DVE 2x mode on every stage
      (tensor_reduce has no fast modes, hence the tt tree).
    - PE bf16 matmuls with on-device-built one-hot matrices (w-major
      is_equal builds, also DVE 2x) scatter-add block sums into
      per-stripe-group PSUM accumulators (f32, exact); per-group
      start/stop lets each group flush mid-stream.
    - per group: one ACT copy flushes PSUM -> bf16, a PE identity
      matmul recombines the four 32-graph quadrants into 128 rows, and
      DVE applies the fused winv = W * (1/count) table, reduces over
      features, adds bias * nonempty -> out rows; 1-stripe final group
      keeps the post-stream tail short.
  Host: concatenate the 8 core outputs.
"""

import sys

for _p in ("/opt/trn_rl_repo",):
    if _p not in sys.path:
        sys.path.insert(0, _p)

import numpy as np
import ml_dtypes
from contextlib import ExitStack

import concourse.bass as bass
import concourse.bacc as bacc
import concourse.tile as tile
from concourse import mybir
from concourse.bass_utils import run_bass_kernel_spmd

P = 128          # partitions
BLK = 16         # nodes per block
D = 15           # feature dim
O = 5            # output dim
GPW = 32         # graphs per window
CPW = 4          # chunks per window (chunk = 128 blocks = 2048 node slots)
SLOTS_W = CPW * P * BLK  # node slots per window

F32 = mybir.dt.float32
BF16 = mybir.dt.bfloat16
BNP = ml_dtypes.bfloat16


# ----------------------------------------------------------------------------
# host planner
# ----------------------------------------------------------------------------

class Plan:
    """Per-run packing plan. All *shape* fields are uniform across cores."""

    def __init__(self, batch, n_cores, G, W=None, b=None):
        self.W = (np.zeros((O, D), np.float32) if W is None
                  else np.asarray(W, np.float32))
        self.b = (np.zeros(O, np.float32) if b is None
                  else np.asarray(b, np.float32))
        batch = np.asarray(batch)
        N = batch.shape[0]
        assert G % (n_cores * P) == 0
        self.G = G
        self.n_cores = n_cores
        self.gpc = G // n_cores                  # graphs per core
        self.nwin = self.gpc // GPW              # windows per core
        self.nstripe = self.gpc // P             # psum stripes per core
        self.nchunk = self.nwin * CPW            # main-stream chunks per core
        self.lslots = self.nwin * SLOTS_W        # node slots per core
        assert self.nwin % self.nstripe == 0
        self.wps = self.nwin // self.nstripe     # windows per stripe
        self.cps = self.nchunk // self.nstripe   # chunks per stripe

        bounds = np.searchsorted(batch, np.arange(G + 1))
        counts = np.diff(bounds).astype(np.int64)
        self.counts = counts
        self.inv = (1.0 / np.maximum(counts, 1.0)).astype(np.float32)
        self.nonempty = (counts > 0).astype(np.float32)

        nblk_g = (counts + BLK - 1) // BLK       # blocks per graph

        # ---- window placement (per core) ----
        self.placements = []
        self.overflow = []
        max_oslots = 8 * BLK
        for c in range(n_cores):
            g0 = c * self.gpc
            placed = []
            oflow = [[] for _ in range(self.nstripe)]
            for w in range(self.nwin):
                gs = [g0 + w * GPW + j for j in range(GPW)
                      if nblk_g[g0 + w * GPW + j] > 0]
                need = {g: int(nblk_g[g]) * BLK for g in gs}
                over = sum(need.values()) - SLOTS_W
                spill = []
                while over > 0:
                    # spill the smallest graph that covers the overage, or
                    # the largest graph if none does (minimizes spilled slots)
                    cand = [g for g in gs if need[g] >= over]
                    g = (min(cand, key=need.get) if cand
                         else max(gs, key=need.get))
                    gs.remove(g); spill.append(g); over -= need[g]
                pos = 0
                for g in gs:
                    placed.append((g, w * SLOTS_W + pos))
                    pos += need[g]
                oflow[w // self.wps].extend(spill)
            for s in range(self.nstripe):
                used = sum(int(nblk_g[g]) * BLK for g in oflow[s])
                max_oslots = max(max_oslots, used)
            self.placements.append(placed)
            self.overflow.append(oflow)

        # overflow blocks per stripe: a single partial chunk (K<128 matmuls)
        # when it fits, whole chunks otherwise
        max_oblk = -(-max_oslots // BLK)
        if max_oblk <= P:
            self.osb = max(8, -(-max_oblk // 8) * 8)
            self.ocps = 1
        else:
            self.osb = P
            self.ocps = -(-max_oblk // P)
        self.osps = self.ocps * self.osb * BLK   # overflow slots per stripe
        self.noch = self.nstripe * self.ocps     # total overflow chunks
        self.bounds = bounds
        self.N = N

    def core_tables(self, c, x):
        """Build per-core input arrays. x is the full [N, D] float32 array."""
        lslots, nchunk = self.lslots, self.nchunk
        g0 = c * self.gpc

        idx = np.full(lslots, -1, dtype=np.int64)
        asg = np.full(lslots // BLK, -1.0, dtype=np.float32)  # local graph/blk
        for g, base in self.placements[c]:
            s0, cnt = int(self.bounds[g]), int(self.counts[g])
            idx[base : base + cnt] = np.arange(s0, s0 + cnt)
            nb = (cnt + BLK - 1) // BLK
            asg[base // BLK : base // BLK + nb] = g - g0

        # overflow stream: per stripe a fixed region of osps slots
        oidx = np.full(self.nstripe * self.osps, -1, dtype=np.int64)
        oasg = np.full(self.nstripe * self.osps // BLK, -1.0, dtype=np.float32)
        for s in range(self.nstripe):
            pos = s * self.osps
            for g in self.overflow[c][s]:
                s0, cnt = int(self.bounds[g]), int(self.counts[g])
                nb = (cnt + BLK - 1) // BLK
                assert pos + nb * BLK <= (s + 1) * self.osps, "overflow overrun"
                oidx[pos : pos + cnt] = np.arange(s0, s0 + cnt)
                oasg[pos // BLK : pos // BLK + nb] = (g - g0) - s * P
                pos += nb * BLK

        def pack(idx_arr, nch, parts):
            # gather, then block t -> (partition t % parts, chunk t // parts),
            # partition-major layout, blocks in natural node-major order
            # (so every halving-tree stage is stride-1 innermost -> DVE 2x)
            out = x[np.clip(idx_arr, 0, self.N - 1)]
            out[idx_arr < 0] = 0.0
            out = out.reshape(nch, parts, BLK * D).transpose(1, 0, 2)
            return np.ascontiguousarray(out.reshape(parts, nch * BLK * D)
                                        .astype(BNP))

        xw = pack(idx, nchunk, P)                 # [P, nchunk*CB] bf16
        xb = pack(oidx, self.noch, self.osb)      # [osb, noch*CB] bf16

        # block t -> partition t%128, chunk t//128; window of chunk m = m//CPW
        t = np.arange(lslots // BLK)
        asgJ = np.full((P, nchunk), -1.0, dtype=np.float32)
        win_base = (t // P // CPW) * GPW
        vals = np.where(asg >= 0, asg - win_base, -1.0)
        asgJ[t % P, t // P] = vals

        asgO = np.full((P, self.noch), -1.0, dtype=np.float32)
        asgO[: self.osb, :] = oasg.reshape(self.noch, self.osb).T

        def stripe_pack(v):
            # graph g (local) -> [partition g%128, col g//128]
            return np.ascontiguousarray(
                v[g0 : g0 + self.gpc].reshape(self.nstripe, P).T.astype(np.float32)
            )

        inv_ps = stripe_pack(self.inv)                       # [P, nstripe] f32
        # fused W x 1/count table: winv[p, o, s, f] = W[o, f] * inv[g(p, s)]
        winv = (inv_ps[:, None, :, None] *
                self.W[None, :, None, :]).astype(BNP)        # [P,O,S,D] bf16
        bne = (stripe_pack(self.nonempty)[:, :, None] *
               self.b[None, None, :]).reshape(P, self.nstripe * O)
        # pad bne to 128 f32 cols so the DMA descriptor is >= 512B
        bne_pad = np.zeros((P, 128), np.float32)
        bne_pad[:, : self.nstripe * O] = bne
        # identity selection matrix for the quadrant recombine:
        # i4[k, q*P + m] = 1.0 iff m == q*GPW + k   (bf16, built on host)
        nq = P // GPW
        i4 = np.zeros((GPW, nq * P), BNP)
        for q in range(nq):
            for k in range(GPW):
                i4[k, q * P + q * GPW + k] = 1.0

        return {
            "xw": xw.reshape(-1),
            "xb": xb.reshape(-1),
            "asgJ": asgJ.astype(BNP),
            "asgO": asgO.astype(BNP),
            "winv": np.ascontiguousarray(winv.reshape(P, -1)),
            "bne": bne_pad,
            "i4": i4,
        }


# ----------------------------------------------------------------------------
# device program
# ----------------------------------------------------------------------------

def build_program(plan, W, b):
    """Build + compile the SPMD Bass program (one program, 8 cores)."""
    nchunk, noch, nstripe = plan.nchunk, plan.noch, plan.nstripe
    cps, wps, osb = plan.cps, plan.wps, plan.osb
    CB = BLK * D     # elements per block (240)

    nc = bacc.Bacc("TRN2", target_bir_lowering=False, debug=False)

    xw = nc.dram_tensor("xw", [P * nchunk * CB], BF16, kind="ExternalInput")
    xb = nc.dram_tensor("xb", [osb * noch * CB], BF16, kind="ExternalInput")
    asgJ = nc.dram_tensor("asgJ", [P, nchunk], BF16, kind="ExternalInput")
    asgO = nc.dram_tensor("asgO", [P, noch], BF16, kind="ExternalInput")
    winv_t = nc.dram_tensor("winv", [P, O * nstripe * D], BF16,
                            kind="ExternalInput")
    bne_t = nc.dram_tensor("bne", [P, 128], F32, kind="ExternalInput")
    i4_t = nc.dram_tensor("i4", [GPW, (P // GPW) * P], BF16,
                          kind="ExternalInput")
    out_t = nc.dram_tensor("out", [plan.gpc * O], F32, kind="ExternalOutput")

    # x tiles: chunks per DMA tile. Tapered: small first tiles so DVE starts
    # early and is never starved, small last tiles so the post-DMA tail is
    # short. Stage-1 of the big middle tiles runs on the (otherwise idle)
    # GPSIMD engine to keep the DVE chain under the DMA roofline.
    KCS = []
    rem = nchunk
    for k in (2, 4, 8, 8, 16, 16):
        kc = min(k, rem)
        if kc:
            KCS.append(kc); rem -= kc
    while rem > 16 + 8 + 4 + 2 + 12:
        KCS.append(min(32, rem - 42)); rem -= KCS[-1]
    for k in (16, 12, 8, 4, 2):
        kc = min(k, rem)
        if kc:
            KCS.append(kc); rem -= kc
    assert sum(KCS) == nchunk and rem == 0
    nquad = P // GPW
    # flush/recombine/projection groups of stripes; the trailing 3+1 split
    # keeps the very last group (one stripe) tiny so the tail is short.
    GROUPS = []
    s = 0
    while s + 4 < nstripe:
        GROUPS.append((s, 4)); s += 4
    if nstripe - s > 1:
        GROUPS.append((s, nstripe - s - 1)); s = nstripe - 1
    GROUPS.append((s, 1))

    with tile.TileContext(nc) as tc, ExitStack() as ctx:
        consts = ctx.enter_context(tc.tile_pool(name="consts", bufs=1))
        xpool = ctx.enter_context(tc.tile_pool(name="xpool", bufs=5))
        spool = ctx.enter_context(tc.tile_pool(name="spool", bufs=2))
        bpool = ctx.enter_context(tc.tile_pool(name="bpool", bufs=1))
        ppool = ctx.enter_context(tc.tile_pool(name="ppool", bufs=2, space="PSUM"))

        def ap_of(handle, offset, pattern):
            return bass.AP(tensor=handle.ap().tensor, offset=offset, ap=pattern)

        def tap(t, offset, pattern):
            return bass.AP(tensor=t.tensor, offset=t.offset + offset, ap=pattern)

        # ---- constant tables (ACT HWDGE ring; keeps SP ring free for x) ----
        # Only asgJ is needed immediately (first one-hot builds); the rest is
        # pinned behind the early x tiles so the x stream owns the DMA bus.
        asgJ_sb = consts.tile([P, nchunk], BF16)
        asgO_sb = consts.tile([P, noch], BF16)
        i4_sb = consts.tile([GPW, nquad * P], BF16)
        winv_sb = consts.tile([P, O * nstripe * D], BF16)
        bne_sb = consts.tile([P, 128], F32)

        # ---- iota tables for one-hot construction (w-major layouts) ----
        # iota_rep[p, w*cps + m] = w   for the main-stream one-hot
        iota_rep = consts.tile([P, GPW * cps], BF16)
        nc.gpsimd.iota(
            iota_rep[:],
            pattern=[[1, GPW], [0, cps]],
            base=0,
            channel_multiplier=0,
            allow_small_or_imprecise_dtypes=True,
        )
        # iota_rep_o[p, w*noch + ch] = w  for the overflow one-hot
        iota_rep_o = consts.tile([P, P * noch], BF16)
        nc.gpsimd.iota(
            iota_rep_o[:],
            pattern=[[1, P], [0, noch]],
            base=0,
            channel_multiplier=0,
            allow_small_or_imprecise_dtypes=True,
        )

        # one-hot arenas (w-major within each stripe so the is_equal build
        # has stride-1 innermost on every operand -> DVE 2x mode):
        #   onehot[p, s*cps*GPW + w*cps + ml] = (asgJ[p, s*cps+ml] == w)
        #   oneO[p, w*noch + ch]              = (asgO[p, ch] == w)
        onehot = bpool.tile([P, nchunk * GPW], BF16)
        oneO = bpool.tile([P, P * noch], BF16)

        def emit_oh(s0, ns):
            # one op builds the one-hot arena for ns stripes (w-major per
            # stripe; every operand has stride-1 innermost -> DVE 2x mode)
            return nc.vector.tensor_tensor(
                out=tap(onehot, s0 * cps * GPW,
                        [onehot.ap[0], [cps * GPW, ns], [cps, GPW], [1, cps]]),
                in0=tap(asgJ_sb, s0 * cps,
                        [asgJ_sb.ap[0], [cps, ns], [0, GPW], [1, cps]]),
                in1=tap(iota_rep, 0,
                        [iota_rep.ap[0], [0, ns], [cps, GPW], [1, cps]]),
                op=mybir.AluOpType.is_equal,
            )

        def emit_oo():
            return nc.vector.tensor_tensor(
                out=tap(oneO, 0,
                        [[oneO.ap[0][0], osb], [noch, P], [1, noch]]),
                in0=tap(asgO_sb, 0,
                        [[asgO_sb.ap[0][0], osb], [0, P], [1, noch]]),
                in1=tap(iota_rep_o, 0,
                        [[iota_rep_o.ap[0][0], osb], [noch, P], [1, noch]]),
                op=mybir.AluOpType.is_equal,
            )

        # ---- block-sum halving tree (DVE tensor_tensor, 2x mode) ----
        # src layout per chunk-column: [D, W] feature-major, W nodes.
        def emit_tree(src, soff, dst, doff, kc, parts, tag):
            # node-major halving tree: every stage adds two contiguous
            # 15-element runs (stride-1 innermost on all operands -> 2x)
            cur, co, w = src, soff, BLK
            last = None
            while w > 1:
                h = w // 2
                if h > 1:
                    nxt = spool.tile([P, kc * D * h], BF16,
                                     tag=f"{tag}{h}", name=f"{tag}{h}")
                    no = 0
                else:
                    nxt, no = dst, doff
                cp = [cur.ap[0][0], parts]
                last = nc.vector.tensor_tensor(
                    out=tap(nxt, no, [[nxt.ap[0][0], parts],
                                      [D * h, kc], [D, h], [1, D]]),
                    in0=tap(cur, co, [cp, [D * w, kc], [D, h], [1, D]]),
                    in1=tap(cur, co + h * D,
                            [cp, [D * w, kc], [D, h], [1, D]]),
                    op=mybir.AluOpType.add,
                )
                cur, co, w = nxt, no, h
            return last

        # ---- overflow stream ----
        xb_sb = bpool.tile([P, noch * CB], BF16)
        Bo = bpool.tile([P, noch * D], BF16)

        # ---- main stream: tapered tiles -> block sums B ----
        B = bpool.tile([P, nchunk * D], BF16)
        KCMAX = max(KCS)
        c0 = 0
        oh_next = 0
        reds = []
        dmas = []
        for ti, KC in enumerate(KCS):
            xt = xpool.tile([P, KCMAX * CB], BF16, tag="xt", name="xt")
            xdma = nc.sync.dma_start(
                out=xt[:, : KC * CB],
                in_=ap_of(xw, c0 * CB, [[nchunk * CB, P], [1, KC * CB]]),
            )
            dmas.append(xdma)
            red = emit_tree(xt, 0, B, c0 * D, KC, P, "st")
            c0 += KC
            reds.append(red)
            # secondary table loads, pinned behind the early x tiles so the
            # x stream owns the DMA bus at the start
            if ti == 0:
                for dma in (
                    nc.scalar.dma_start(out=asgJ_sb[:], in_=asgJ.ap()),
                    nc.scalar.dma_start(out=asgO_sb[:], in_=asgO.ap()),
                    nc.scalar.dma_start(
                        out=xb_sb[:osb, :],
                        in_=ap_of(xb, 0, [[noch * CB, osb], [1, noch * CB]]),
                    ),
                ):
                    tile.add_dep_helper(dma.ins, dmas[0].ins, sync=False,
                                        reason="table loads behind x tiles")
            if ti == 3:
                for dma in (
                    nc.scalar.dma_start(out=i4_sb[:], in_=i4_t.ap()),
                    nc.scalar.dma_start(out=winv_sb[:], in_=winv_t.ap()),
                    nc.scalar.dma_start(out=bne_sb[:], in_=bne_t.ap()),
                ):
                    tile.add_dep_helper(dma.ins, dmas[3].ins, sync=False,
                                        reason="table loads behind x tiles")
            # Pack the small side-jobs behind the early tiles' tree work so
            # the scheduler cannot hoist them ahead of the x pipeline.
            if ti == 0:
                oh = emit_oh(0, nstripe // 2)
                tile.add_dep_helper(oh.ins, red.ins, sync=False,
                                    reason="onehot build in early DVE idle")
            if ti == min(2, len(KCS) - 1):
                bo = emit_tree(xb_sb, 0, Bo, 0, noch, osb, "ob")
                tile.add_dep_helper(bo.ins, red.ins, sync=False,
                                    reason="Bo tree in early DVE idle")
            if ti == min(3, len(KCS) - 1):
                tile.add_dep_helper(emit_oo().ins, red.ins, sync=False,
                                    reason="oneO build in early DVE idle")
            if ti == min(5, len(KCS) - 1):
                oh = emit_oh(nstripe // 2, nstripe - nstripe // 2)
                tile.add_dep_helper(oh.ins, red.ins, sync=False,
                                    reason="onehot half 2 in mid-stream idle")

        # ---- routing matmuls, grouped by stripe-group ----
        # Each group accumulates its stripes into ONE combined quadrant PSUM
        # tile [GPW, nquad*qs*D] (single bank; quadrant q owns columns
        # q*qs*D..): one start=True opener per group clears the bank, all
        # other matmuls accumulate disjoint regions. Per-group start/stop
        # lets the flush/recombine/projection pipeline run mid-stream
        # instead of trailing the whole x stream.
        QSMAX = max(qs for _, qs in GROUPS)
        sums_g = [bpool.tile([GPW, nquad * QSMAX * D], BF16, name=f"sums{g}")
                  for g in range(len(GROUPS))]
        tmp = bpool.tile([P, len(GROUPS) * O * QSMAX * D], F32)
        proj = bpool.tile([P, nstripe * O], F32)
        outv = bpool.tile([P, nstripe * O], F32)

        for g, (s0, qs) in enumerate(GROUPS):
            qsum = ppool.tile([GPW, nquad * QSMAX * D], F32,
                              tag="qsum", name=f"qsum{g}")
            opener = None
            last_mm = None
            for s in range(s0, s0 + qs):
                for q in range(nquad):
                    col = (q * qs + (s - s0)) * D
                    mms = []
                    for j in range(CPW):
                        m = (s * wps + q) * CPW + j
                        ml = m - s * cps
                        mms.append(nc.tensor.matmul(
                            out=qsum[:, col : col + D],
                            lhsT=tap(onehot, s * cps * GPW + ml,
                                     [onehot.ap[0], [cps, GPW]]),
                            rhs=B[:, m * D : (m + 1) * D],
                            start=(opener is None and not mms),
                            stop=False,
                            tile_position=(0, 0),
                            skip_group_check=True,
                        ))
                    glast = (s == s0 + qs - 1) and (q == nquad - 1)
                    for oc in range(plan.ocps):
                        ch = s * plan.ocps + oc
                        mms.append(nc.tensor.matmul(
                            out=qsum[:, col : col + D],
                            lhsT=tap(oneO, ch + q * GPW * noch,
                                     [[oneO.ap[0][0], osb], [noch, GPW]]),
                            rhs=Bo[:osb, ch * D : (ch + 1) * D],
                            start=False,
                            stop=(glast and oc == plan.ocps - 1),
                            tile_position=(0, 0),
                            skip_group_check=True,
                        ))
                    if opener is None:
                        opener = mms[0]
                        mms = mms[1:]
                    for mm in mms:
                        tile.add_dep_helper(mm.ins, opener.ins, sync=False,
                                            reason="psum opener first")
                    last_mm = mms[-1] if mms else opener

            # flush: one ACT copy, PSUM f32 -> SBUF bf16 (1/count is in winv)
            nc.scalar.copy(
                out=sums_g[g][:, : nquad * qs * D],
                in_=qsum[:, : nquad * qs * D],
            )

            # recombine quadrants -> pall [P, qs*D] (PE, bf16 exact)
            pall = ppool.tile([P, QSMAX * D], F32, tag="pall", name=f"pall{g}")
            for q in range(nquad):
                nc.tensor.matmul(
                    out=pall[:, : qs * D],
                    lhsT=i4_sb[:, q * P : (q + 1) * P],
                    rhs=sums_g[g][:, q * qs * D : (q + 1) * qs * D],
                    start=(q == 0),
                    stop=(q == nquad - 1),
                    tile_position=(0, 0),
                    skip_group_check=True,
                )
            # projection: tmp[p,o,s,f] = mean * W[o,f]*inv; reduce f; + bias
            lc = (s0 + qs) * cps - 1     # last chunk this group consumes
            ci = 0
            for ti2, kc2 in enumerate(KCS):
                ci += kc2
                if lc < ci:
                    gate = reds[ti2]
                    break
            toff = g * O * QSMAX * D
            tmp_tt = nc.vector.tensor_tensor(
                out=tap(tmp, toff, [tmp.ap[0], [qs * D, O], [D, qs], [1, D]]),
                in0=tap(pall, 0, [pall.ap[0], [0, O], [D, qs], [1, D]]),
                in1=tap(winv_sb, s0 * D,
                        [winv_sb.ap[0], [nstripe * D, O], [D, qs], [1, D]]),
                op=mybir.AluOpType.mult,
            )
            tile.add_dep_helper(tmp_tt.ins, gate.ins, sync=False,
                                reason="slot group chain at data readiness")
            nc.vector.tensor_reduce(
                out=tap(proj, s0 * O, [proj.ap[0], [1, O], [O, qs]]),
                in_=tap(tmp, toff, [tmp.ap[0], [qs * D, O], [D, qs], [1, D]]),
                axis=mybir.AxisListType.X,
                op=mybir.AluOpType.add,
            )
            nc.vector.tensor_tensor(
                out=outv[:, s0 * O : (s0 + qs) * O],
                in0=proj[:, s0 * O : (s0 + qs) * O],
                in1=bne_sb[:, s0 * O : (s0 + qs) * O],
                op=mybir.AluOpType.add,
            )
            (nc.sync if g % 2 == 0 else nc.scalar).dma_start(
                out=ap_of(out_t, s0 * P * O, [[O, P], [P * O, qs], [1, O]]),
                in_=outv[:, s0 * O : (s0 + qs) * O],
            )

    nc.compile()
    return nc


# ----------------------------------------------------------------------------
# entry point
# ----------------------------------------------------------------------------

_CACHE = {}
_LAST_RESULTS = None


def kernel(x, batch, W, b):
    global _LAST_RESULTS
    x = np.asarray(x, dtype=np.float32)
    batch = np.asarray(batch)
    W = np.asarray(W, dtype=np.float32)
    b = np.asarray(b, dtype=np.float32)

    n_cores = 8
    G = 16384
    plan = Plan(batch, n_cores, G, W, b)

    key = (plan.lslots, plan.nchunk, plan.noch, plan.osps)
    if key not in _CACHE:
        _CACHE[key] = build_program(plan, W, b)
    nc = _CACHE[key]

    in_maps = [plan.core_tables(c, x) for c in range(n_cores)]

    def _run():
        return run_bass_kernel_spmd(nc, in_maps, core_ids=list(range(n_cores)))

    try:
        res = _run()
    except ModuleNotFoundError:
        # BASS_TRACE was set but this container lacks the axon NTFF profiling
        # hook (antenv.axon_hooks) — retry with tracing disabled.
        import os
        os.environ["BASS_NEVER_TRACE"] = "1"
        res = _run()
    except Exception as e:  # transient device/terminal failure -> one retry
        if not any(k in str(e) for k in ("UNAVAILABLE", "UNRECOVERABLE")):
            raise
        import time as _time
        _time.sleep(10.0)
        res = _run()
    _LAST_RESULTS = res
    out = np.concatenate(
        [res.results[c]["out"].reshape(plan.gpc, O) for c in range(n_cores)],
        axis=0,
    )
    return out.astype(np.float32)


if __name__ == "__main__":
    # tiny smoke test of the planner only
    rng = np.random.default_rng(0)
    N, G = 400_000, 16384
    batch = np.sort(rng.integers(0, G, N))
    x = rng.standard_normal((N, D), dtype=np.float32)
    plan = Plan(batch, 8, G)
    print("lslots", plan.lslots, "nchunk", plan.nchunk, "osps", plan.osps)
    t = plan.core_tables(0, x)
    for k, v in t.items():
        print(k, v.shape, v.dtype)


# revision 29
# speedup vs baseline: 1.8632x; 1.0305x over previous
"""Trainium2 Bass kernel for fused segment-mean + linear projection.

Reference computation (for x[N,15], sorted batch[N] in [0,G), W[5,15], b[5]):
    sums[g]  = segment_sum(x, batch)          # [G, 15]
    counts[g]= segment_sum(1, batch)          # [G]
    mean     = sums / max(counts, 1)
    out      = where(counts > 0, mean @ W.T + b, 0)   # [G, 5]

Strategy (8 NeuronCores, data parallel over contiguous graph-id ranges):
  Host (index-only preprocessing; x is repacked and cast to bf16 --
  the 2e-2 correctness budget dwarfs bf16 quantization error, and it
  halves the HBM traffic this memory-bound kernel is limited by):
    - each core owns G/8 consecutive graphs; its nodes are repacked into
      "windows" of GPW=32 graphs x 8192 node-slots (4 chunks of 128
      16-node blocks), each graph zero-padded to a 16-node multiple so
      every block belongs to exactly one graph. Graphs that do not fit
      their window spill whole into a small per-stripe overflow stream.
    - the packed stream is bf16 PARTITION-MAJOR: each SBUF partition's
      chunk range is contiguous in DRAM, so DMA descriptors stay >= 512B
      (full 360 GB/s; no small-descriptor penalty) at half the bytes.
    - all program shapes / the matmul schedule are data-independent, so
      one SPMD program serves all 8 cores; per-core data differs only in
      the input tables.
  Device (per core):
    - DMA bf16 x tiles (tapered sizes); block sums via a 4-stage DVE
      tensor_tensor halving tree (16->8->4->2->1 nodes). Blocks are
      node-major so every stage adds two contiguous 15-element runs:
      all operands are 2-byte stride-1 -> DVE 2x mode on every stage
      (tensor_reduce has no fast modes, hence the tt tree).
    - PE bf16 matmuls with one-hot matrices scatter-add block sums
      into per-stripe-group PSUM accumulators (f32, exact); the one-hot
      arenas are built by GPSIMD local_scatter from host-made int16
      index tables (zero DVE cost); per-group start/stop lets each
      group flush mid-stream.
    - per group: one ACT copy flushes PSUM -> bf16, a PE identity
      matmul recombines the four 32-graph quadrants into 128 rows, and
      DVE applies the fused winv = W * (1/count) table, reduces over
      features, adds bias * nonempty -> out rows; 1-stripe final group
      keeps the post-stream tail short.
  Host: concatenate the 8 core outputs.
"""

import sys

for _p in ("/opt/trn_rl_repo",):
    if _p not in sys.path:
        sys.path.insert(0, _p)

import numpy as np
import ml_dtypes
from contextlib import ExitStack

import concourse.bass as bass
import concourse.bacc as bacc
import concourse.tile as tile
from concourse import mybir
from concourse.bass_utils import run_bass_kernel_spmd

P = 128          # partitions
BLK = 16         # nodes per block
D = 15           # feature dim
O = 5            # output dim
GPW = 32         # graphs per window
CPW = 4          # chunks per window (chunk = 128 blocks = 2048 node slots)
SLOTS_W = CPW * P * BLK  # node slots per window

F32 = mybir.dt.float32
BF16 = mybir.dt.bfloat16
BNP = ml_dtypes.bfloat16


# ----------------------------------------------------------------------------
# host planner
# ----------------------------------------------------------------------------

class Plan:
    """Per-run packing plan. All *shape* fields are uniform across cores."""

    def __init__(self, batch, n_cores, G, W=None, b=None):
        self.W = (np.zeros((O, D), np.float32) if W is None
                  else np.asarray(W, np.float32))
        self.b = (np.zeros(O, np.float32) if b is None
                  else np.asarray(b, np.float32))
        batch = np.asarray(batch)
        N = batch.shape[0]
        assert G % (n_cores * P) == 0
        self.G = G
        self.n_cores = n_cores
        self.gpc = G // n_cores                  # graphs per core
        self.nwin = self.gpc // GPW              # windows per core
        self.nstripe = self.gpc // P             # psum stripes per core
        self.nchunk = self.nwin * CPW            # main-stream chunks per core
        self.lslots = self.nwin * SLOTS_W        # node slots per core
        assert self.nwin % self.nstripe == 0
        self.wps = self.nwin // self.nstripe     # windows per stripe
        self.cps = self.nchunk // self.nstripe   # chunks per stripe

        bounds = np.searchsorted(batch, np.arange(G + 1))
        counts = np.diff(bounds).astype(np.int64)
        self.counts = counts
        self.inv = (1.0 / np.maximum(counts, 1.0)).astype(np.float32)
        self.nonempty = (counts > 0).astype(np.float32)

        nblk_g = (counts + BLK - 1) // BLK       # blocks per graph

        # ---- window placement (per core) ----
        self.placements = []
        self.overflow = []
        max_oslots = 8 * BLK
        for c in range(n_cores):
            g0 = c * self.gpc
            placed = []
            oflow = [[] for _ in range(self.nstripe)]
            for w in range(self.nwin):
                gs = [g0 + w * GPW + j for j in range(GPW)
                      if nblk_g[g0 + w * GPW + j] > 0]
                need = {g: int(nblk_g[g]) * BLK for g in gs}
                over = sum(need.values()) - SLOTS_W
                spill = []
                while over > 0:
                    # spill the smallest graph that covers the overage, or
                    # the largest graph if none does (minimizes spilled slots)
                    cand = [g for g in gs if need[g] >= over]
                    g = (min(cand, key=need.get) if cand
                         else max(gs, key=need.get))
                    gs.remove(g); spill.append(g); over -= need[g]
                pos = 0
                for g in gs:
                    placed.append((g, w * SLOTS_W + pos))
                    pos += need[g]
                oflow[w // self.wps].extend(spill)
            for s in range(self.nstripe):
                used = sum(int(nblk_g[g]) * BLK for g in oflow[s])
                max_oslots = max(max_oslots, used)
            self.placements.append(placed)
            self.overflow.append(oflow)

        # overflow blocks per stripe: a single partial chunk (K<128 matmuls)
        # when it fits, whole chunks otherwise
        max_oblk = -(-max_oslots // BLK)
        if max_oblk <= P:
            self.osb = max(8, -(-max_oblk // 8) * 8)
            self.ocps = 1
        else:
            self.osb = P
            self.ocps = -(-max_oblk // P)
        self.osps = self.ocps * self.osb * BLK   # overflow slots per stripe
        self.noch = self.nstripe * self.ocps     # total overflow chunks
        self.bounds = bounds
        self.N = N

    def core_tables(self, c, x):
        """Build per-core input arrays. x is the full [N, D] float32 array."""
        lslots, nchunk = self.lslots, self.nchunk
        g0 = c * self.gpc

        idx = np.full(lslots, -1, dtype=np.int64)
        asg = np.full(lslots // BLK, -1.0, dtype=np.float32)  # local graph/blk
        for g, base in self.placements[c]:
            s0, cnt = int(self.bounds[g]), int(self.counts[g])
            idx[base : base + cnt] = np.arange(s0, s0 + cnt)
            nb = (cnt + BLK - 1) // BLK
            asg[base // BLK : base // BLK + nb] = g - g0

        # overflow stream: per stripe a fixed region of osps slots
        oidx = np.full(self.nstripe * self.osps, -1, dtype=np.int64)
        oasg = np.full(self.nstripe * self.osps // BLK, -1.0, dtype=np.float32)
        for s in range(self.nstripe):
            pos = s * self.osps
            for g in self.overflow[c][s]:
                s0, cnt = int(self.bounds[g]), int(self.counts[g])
                nb = (cnt + BLK - 1) // BLK
                assert pos + nb * BLK <= (s + 1) * self.osps, "overflow overrun"
                oidx[pos : pos + cnt] = np.arange(s0, s0 + cnt)
                oasg[pos // BLK : pos // BLK + nb] = (g - g0) - s * P
                pos += nb * BLK

        def pack(idx_arr, nch, parts):
            # gather, then block t -> (partition t % parts, chunk t // parts),
            # partition-major layout, blocks in natural node-major order
            # (so every halving-tree stage is stride-1 innermost -> DVE 2x)
            out = x[np.clip(idx_arr, 0, self.N - 1)]
            out[idx_arr < 0] = 0.0
            out = out.reshape(nch, parts, BLK * D).transpose(1, 0, 2)
            return np.ascontiguousarray(out.reshape(parts, nch * BLK * D)
                                        .astype(BNP))

        xw = pack(idx, nchunk, P)                 # [P, nchunk*CB] bf16
        xb = pack(oidx, self.noch, self.osb)      # [osb, noch*CB] bf16

        # block t -> partition t%128, chunk t//128; window of chunk m = m//CPW
        # idxJ[p, m]: scatter index into the one-hot arena (w-major per
        # stripe, 1024-column sections of 2 stripes), -1 for padding blocks
        t = np.arange(lslots // BLK)
        win_base = (t // P // CPW) * GPW
        w = np.where(asg >= 0, asg - win_base, -1.0).astype(np.int64)
        m = t // P
        s = m // self.cps
        iJ = np.where(w >= 0,
                      (s % 2) * (self.cps * GPW) + w * self.cps + m % self.cps,
                      -1)
        idxJ = np.full((P, nchunk), -1, dtype=np.int16)
        idxJ[t % P, m] = iJ.astype(np.int16)

        # idxO[p, sec*noch + ch]: index into the overflow one-hot arena
        # (w*noch + ch layout, 1024-column sections of wspan=1024//noch)
        oasgT = np.full((P, self.noch), -1, dtype=np.int64)
        oasgT[: self.osb, :] = oasg.reshape(self.noch, self.osb).T.astype(
            np.int64)
        wspan = 1024 // self.noch
        nsec = P // wspan
        ch = np.arange(self.noch)[None, :]
        idxO = np.full((P, nsec * self.noch), -1, dtype=np.int16)
        for sec in range(nsec):
            wl = oasgT - sec * wspan
            ok = (wl >= 0) & (wl < wspan)
            idxO[:, sec * self.noch : (sec + 1) * self.noch] = np.where(
                ok, wl * self.noch + ch, -1).astype(np.int16)

        def stripe_pack(v):
            # graph g (local) -> [partition g%128, col g//128]
            return np.ascontiguousarray(
                v[g0 : g0 + self.gpc].reshape(self.nstripe, P).T.astype(np.float32)
            )

        inv_ps = stripe_pack(self.inv)                       # [P, nstripe] f32
        # fused W x 1/count table: winv[p, o, s, f] = W[o, f] * inv[g(p, s)]
        winv = (inv_ps[:, None, :, None] *
                self.W[None, :, None, :]).astype(BNP)        # [P,O,S,D] bf16
        bne = (stripe_pack(self.nonempty)[:, :, None] *
               self.b[None, None, :]).reshape(P, self.nstripe * O)
        # pad bne to 128 f32 cols so the DMA descriptor is >= 512B
        bne_pad = np.zeros((P, 128), np.float32)
        bne_pad[:, : self.nstripe * O] = bne
        # identity selection matrix for the quadrant recombine:
        # i4[k, q*P + m] = 1.0 iff m == q*GPW + k   (bf16, built on host)
        nq = P // GPW
        i4 = np.zeros((GPW, nq * P), BNP)
        for q in range(nq):
            for k in range(GPW):
                i4[k, q * P + q * GPW + k] = 1.0

        return {
            "xw": xw.reshape(-1),
            "xb": xb.reshape(-1),
            "idxJ": idxJ,
            "idxO": idxO,
            "winv": np.ascontiguousarray(winv.reshape(P, -1)),
            "bne": bne_pad,
            "i4": i4,
        }


# ----------------------------------------------------------------------------
# device program
# ----------------------------------------------------------------------------

def build_program(plan, W, b):
    """Build + compile the SPMD Bass program (one program, 8 cores)."""
    nchunk, noch, nstripe = plan.nchunk, plan.noch, plan.nstripe
    cps, wps, osb = plan.cps, plan.wps, plan.osb
    CB = BLK * D     # elements per block (240)

    nc = bacc.Bacc("TRN2", target_bir_lowering=False, debug=False)

    xw = nc.dram_tensor("xw", [P * nchunk * CB], BF16, kind="ExternalInput")
    xb = nc.dram_tensor("xb", [osb * noch * CB], BF16, kind="ExternalInput")
    nsec_o = P // (1024 // noch)
    idxJ = nc.dram_tensor("idxJ", [P, nchunk], mybir.dt.int16,
                          kind="ExternalInput")
    idxO = nc.dram_tensor("idxO", [P, nsec_o * noch], mybir.dt.int16,
                          kind="ExternalInput")
    winv_t = nc.dram_tensor("winv", [P, O * nstripe * D], BF16,
                            kind="ExternalInput")
    bne_t = nc.dram_tensor("bne", [P, 128], F32, kind="ExternalInput")
    i4_t = nc.dram_tensor("i4", [GPW, (P // GPW) * P], BF16,
                          kind="ExternalInput")
    out_t = nc.dram_tensor("out", [plan.gpc * O], F32, kind="ExternalOutput")

    # x tiles: chunks per DMA tile. Tapered: small first tiles so DVE starts
    # early and is never starved, small last tiles so the post-DMA tail is
    # short. Stage-1 of the big middle tiles runs on the (otherwise idle)
    # GPSIMD engine to keep the DVE chain under the DMA roofline.
    KCS = []
    rem = nchunk
    TAIL = (16, 12, 8, 4, 2)
    for k in (2, 4, 8, 8, 16, 16):
        kc = min(k, rem)
        if kc:
            KCS.append(kc); rem -= kc
    while rem > sum(TAIL):
        KCS.append(min(32, rem - sum(TAIL))); rem -= KCS[-1]
    for k in TAIL:
        kc = min(k, rem)
        if kc:
            KCS.append(kc); rem -= kc
    assert sum(KCS) == nchunk and rem == 0
    nquad = P // GPW
    # flush/recombine/projection groups of stripes; the trailing 3+1 split
    # keeps the very last group (one stripe) tiny so the tail is short.
    GROUPS = []
    s = 0
    while s + 4 < nstripe:
        GROUPS.append((s, 4)); s += 4
    if nstripe - s > 1:
        GROUPS.append((s, nstripe - s - 1)); s = nstripe - 1
    GROUPS.append((s, 1))

    with tile.TileContext(nc) as tc, ExitStack() as ctx:
        consts = ctx.enter_context(tc.tile_pool(name="consts", bufs=1))
        xpool = ctx.enter_context(tc.tile_pool(name="xpool", bufs=5))
        spool = ctx.enter_context(tc.tile_pool(name="spool", bufs=2))
        bpool = ctx.enter_context(tc.tile_pool(name="bpool", bufs=1))
        ppool = ctx.enter_context(tc.tile_pool(name="ppool", bufs=2, space="PSUM"))

        def ap_of(handle, offset, pattern):
            return bass.AP(tensor=handle.ap().tensor, offset=offset, ap=pattern)

        def tap(t, offset, pattern):
            return bass.AP(tensor=t.tensor, offset=t.offset + offset, ap=pattern)

        # ---- constant tables (ACT HWDGE ring; keeps SP ring free for x) ----
        # Only asgJ is needed immediately (first one-hot builds); the rest is
        # pinned behind the early x tiles so the x stream owns the DMA bus.
        idxJ_sb = consts.tile([P, nchunk], mybir.dt.int16)
        idxO_sb = consts.tile([P, nsec_o * noch], mybir.dt.int16)
        i4_sb = consts.tile([GPW, nquad * P], BF16)
        winv_sb = consts.tile([P, O * nstripe * D], BF16)
        bne_sb = consts.tile([P, 128], F32)

        # all-ones data row for the one-hot scatters
        ones_sb = consts.tile([P, cps * 2], BF16)
        nc.gpsimd.memset(ones_sb[:], 1.0)

        # one-hot arenas, built by GPSIMD local_scatter (dst is zeroed by
        # the instruction itself; negative indices = padding are ignored):
        #   onehot[p, s*cps*GPW + w*cps + ml] = 1 at idxJ positions
        #   oneO[p, w*noch + ch]              = 1 at idxO positions
        onehot = bpool.tile([P, nchunk * GPW], BF16)
        oneO = bpool.tile([P, P * noch], BF16)
        SEC = 2 * cps * GPW          # columns per scatter section (2 stripes)
        assert SEC == 1024 and SEC * 32 < 2 ** 16

        def emit_oh(sec):
            nix = 2 * cps            # chunks per section
            return nc.gpsimd.local_scatter(
                out_ap=onehot[:, sec * SEC : (sec + 1) * SEC],
                data_ap=ones_sb[:, :nix],
                idxs_ap=idxJ_sb[:, sec * nix : (sec + 1) * nix],
                channels=P,
                num_elems=SEC,
                num_idxs=nix,
            )

        def emit_oo(sec):
            return nc.gpsimd.local_scatter(
                out_ap=oneO[:, sec * 1024 : (sec + 1) * 1024],
                data_ap=ones_sb[:, :noch],
                idxs_ap=idxO_sb[:, sec * noch : (sec + 1) * noch],
                channels=P,
                num_elems=1024,
                num_idxs=noch,
            )

        # ---- block-sum halving tree (DVE tensor_tensor, 2x mode) ----
        # src layout per chunk-column: [D, W] feature-major, W nodes.
        def emit_tree(src, soff, dst, doff, kc, parts, tag):
            # node-major halving tree: every stage adds two contiguous
            # 15-element runs (stride-1 innermost on all operands -> 2x)
            cur, co, w = src, soff, BLK
            last = None
            while w > 1:
                h = w // 2
                if h > 1:
                    nxt = spool.tile([P, kc * D * h], BF16,
                                     tag=f"{tag}{h}", name=f"{tag}{h}")
                    no = 0
                else:
                    nxt, no = dst, doff
                cp = [cur.ap[0][0], parts]
                last = nc.vector.tensor_tensor(
                    out=tap(nxt, no, [[nxt.ap[0][0], parts],
                                      [D * h, kc], [D, h], [1, D]]),
                    in0=tap(cur, co, [cp, [D * w, kc], [D, h], [1, D]]),
                    in1=tap(cur, co + h * D,
                            [cp, [D * w, kc], [D, h], [1, D]]),
                    op=mybir.AluOpType.add,
                )
                cur, co, w = nxt, no, h
            return last

        # ---- overflow stream ----
        xb_sb = bpool.tile([P, noch * CB], BF16)
        Bo = bpool.tile([P, noch * D], BF16)

        # ---- main stream: tapered tiles -> block sums B ----
        B = bpool.tile([P, nchunk * D], BF16)
        KCMAX = max(KCS)
        c0 = 0
        oh_next = 0
        reds = []
        dmas = []
        for ti, KC in enumerate(KCS):
            xt = xpool.tile([P, KCMAX * CB], BF16, tag="xt", name="xt")
            xdma = nc.sync.dma_start(
                out=xt[:, : KC * CB],
                in_=ap_of(xw, c0 * CB, [[nchunk * CB, P], [1, KC * CB]]),
            )
            dmas.append(xdma)
            red = emit_tree(xt, 0, B, c0 * D, KC, P, "st")
            c0 += KC
            reds.append(red)
            # secondary table loads, pinned behind the early x tiles so the
            # x stream owns the DMA bus at the start
            if ti == 0:
                for dma in (
                    nc.scalar.dma_start(out=idxJ_sb[:], in_=idxJ.ap()),
                    nc.scalar.dma_start(out=idxO_sb[:], in_=idxO.ap()),
                    nc.scalar.dma_start(
                        out=xb_sb[:osb, :],
                        in_=ap_of(xb, 0, [[noch * CB, osb], [1, noch * CB]]),
                    ),
                ):
                    tile.add_dep_helper(dma.ins, dmas[0].ins, sync=False,
                                        reason="table loads behind x tiles")
            if ti == 3:
                for dma in (
                    nc.scalar.dma_start(out=i4_sb[:], in_=i4_t.ap()),
                    nc.scalar.dma_start(out=winv_sb[:], in_=winv_t.ap()),
                    nc.scalar.dma_start(out=bne_sb[:], in_=bne_t.ap()),
                ):
                    tile.add_dep_helper(dma.ins, dmas[3].ins, sync=False,
                                        reason="table loads behind x tiles")
            # Pack the small side-jobs behind the early tiles' tree work so
            # the scheduler cannot hoist them ahead of the x pipeline.
            if ti == 0:
                for sec in range(nstripe // 2):
                    emit_oh(sec)
                for sec in range(nsec_o):
                    emit_oo(sec)
            if ti == min(2, len(KCS) - 1):
                bo = emit_tree(xb_sb, 0, Bo, 0, noch, osb, "ob")
                tile.add_dep_helper(bo.ins, red.ins, sync=False,
                                    reason="Bo tree in early DVE idle")

        # ---- routing matmuls, grouped by stripe-group ----
        # Each group accumulates its stripes into ONE combined quadrant PSUM
        # tile [GPW, nquad*qs*D] (single bank; quadrant q owns columns
        # q*qs*D..): one start=True opener per group clears the bank, all
        # other matmuls accumulate disjoint regions. Per-group start/stop
        # lets the flush/recombine/projection pipeline run mid-stream
        # instead of trailing the whole x stream.
        QSMAX = max(qs for _, qs in GROUPS)
        sums_g = [bpool.tile([GPW, nquad * QSMAX * D], BF16, name=f"sums{g}")
                  for g in range(len(GROUPS))]
        tmp = bpool.tile([P, len(GROUPS) * O * QSMAX * D], F32)
        proj = bpool.tile([P, nstripe * O], F32)
        outv = bpool.tile([P, nstripe * O], F32)

        for g, (s0, qs) in enumerate(GROUPS):
            qsum = ppool.tile([GPW, nquad * QSMAX * D], F32,
                              tag="qsum", name=f"qsum{g}")
            opener = None
            last_mm = None
            for s in range(s0, s0 + qs):
                for q in range(nquad):
                    col = (q * qs + (s - s0)) * D
                    mms = []
                    for j in range(CPW):
                        m = (s * wps + q) * CPW + j
                        ml = m - s * cps
                        mms.append(nc.tensor.matmul(
                            out=qsum[:, col : col + D],
                            lhsT=tap(onehot, s * cps * GPW + ml,
                                     [onehot.ap[0], [cps, GPW]]),
                            rhs=B[:, m * D : (m + 1) * D],
                            start=(opener is None and not mms),
                            stop=False,
                            tile_position=(0, 0),
                            skip_group_check=True,
                        ))
                    glast = (s == s0 + qs - 1) and (q == nquad - 1)
                    for oc in range(plan.ocps):
                        ch = s * plan.ocps + oc
                        mms.append(nc.tensor.matmul(
                            out=qsum[:, col : col + D],
                            lhsT=tap(oneO, ch + q * GPW * noch,
                                     [[oneO.ap[0][0], osb], [noch, GPW]]),
                            rhs=Bo[:osb, ch * D : (ch + 1) * D],
                            start=False,
                            stop=(glast and oc == plan.ocps - 1),
                            tile_position=(0, 0),
                            skip_group_check=True,
                        ))
                    if opener is None:
                        opener = mms[0]
                        mms = mms[1:]
                    for mm in mms:
                        tile.add_dep_helper(mm.ins, opener.ins, sync=False,
                                            reason="psum opener first")
                    last_mm = mms[-1] if mms else opener

            # flush: PSUM f32 -> SBUF bf16 (1/count is in winv). The last
            # group flushes on DVE (idle at that point) to cut a cross-
            # engine hop off the tail; earlier groups use the idle ACT.
            if g >= len(GROUPS) - 2:
                nc.vector.tensor_copy(
                    out=sums_g[g][:, : nquad * qs * D],
                    in_=qsum[:, : nquad * qs * D],
                )
            else:
                nc.scalar.copy(
                    out=sums_g[g][:, : nquad * qs * D],
                    in_=qsum[:, : nquad * qs * D],
                )

            # recombine quadrants -> pall [P, qs*D] (PE, bf16 exact)
            pall = ppool.tile([P, QSMAX * D], F32, tag="pall", name=f"pall{g}")
            for q in range(nquad):
                nc.tensor.matmul(
                    out=pall[:, : qs * D],
                    lhsT=i4_sb[:, q * P : (q + 1) * P],
                    rhs=sums_g[g][:, q * qs * D : (q + 1) * qs * D],
                    start=(q == 0),
                    stop=(q == nquad - 1),
                    tile_position=(0, 0),
                    skip_group_check=True,
                )
            # projection: tmp[p,o,s,f] = mean * W[o,f]*inv; reduce f; + bias
            lc = (s0 + qs) * cps - 1     # last chunk this group consumes
            ci = 0
            for ti2, kc2 in enumerate(KCS):
                ci += kc2
                if lc < ci:
                    gate = reds[ti2]
                    break
            toff = g * O * QSMAX * D
            tmp_tt = nc.vector.tensor_tensor(
                out=tap(tmp, toff, [tmp.ap[0], [qs * D, O], [D, qs], [1, D]]),
                in0=tap(pall, 0, [pall.ap[0], [0, O], [D, qs], [1, D]]),
                in1=tap(winv_sb, s0 * D,
                        [winv_sb.ap[0], [nstripe * D, O], [D, qs], [1, D]]),
                op=mybir.AluOpType.mult,
            )
            tile.add_dep_helper(tmp_tt.ins, gate.ins, sync=False,
                                reason="slot group chain at data readiness")
            nc.vector.tensor_reduce(
                out=tap(proj, s0 * O, [proj.ap[0], [1, O], [O, qs]]),
                in_=tap(tmp, toff, [tmp.ap[0], [qs * D, O], [D, qs], [1, D]]),
                axis=mybir.AxisListType.X,
                op=mybir.AluOpType.add,
            )
            nc.vector.tensor_tensor(
                out=outv[:, s0 * O : (s0 + qs) * O],
                in0=proj[:, s0 * O : (s0 + qs) * O],
                in1=bne_sb[:, s0 * O : (s0 + qs) * O],
                op=mybir.AluOpType.add,
            )
            (nc.sync if g % 2 == 0 else nc.scalar).dma_start(
                out=ap_of(out_t, s0 * P * O, [[O, P], [P * O, qs], [1, O]]),
                in_=outv[:, s0 * O : (s0 + qs) * O],
            )

    nc.compile()
    return nc


# ----------------------------------------------------------------------------
# entry point
# ----------------------------------------------------------------------------

_CACHE = {}
_LAST_RESULTS = None


def kernel(x, batch, W, b):
    global _LAST_RESULTS
    x = np.asarray(x, dtype=np.float32)
    batch = np.asarray(batch)
    W = np.asarray(W, dtype=np.float32)
    b = np.asarray(b, dtype=np.float32)

    n_cores = 8
    G = 16384
    plan = Plan(batch, n_cores, G, W, b)

    key = (plan.lslots, plan.nchunk, plan.noch, plan.osps)
    if key not in _CACHE:
        _CACHE[key] = build_program(plan, W, b)
    nc = _CACHE[key]

    in_maps = [plan.core_tables(c, x) for c in range(n_cores)]

    def _run():
        return run_bass_kernel_spmd(nc, in_maps, core_ids=list(range(n_cores)))

    try:
        res = _run()
    except ModuleNotFoundError:
        # BASS_TRACE was set but this container lacks the axon NTFF profiling
        # hook (antenv.axon_hooks) — retry with tracing disabled.
        import os
        os.environ["BASS_NEVER_TRACE"] = "1"
        res = _run()
    except Exception as e:  # transient device/terminal failure -> one retry
        if not any(k in str(e) for k in ("UNAVAILABLE", "UNRECOVERABLE")):
            raise
        import time as _time
        _time.sleep(10.0)
        res = _run()
    _LAST_RESULTS = res
    out = np.concatenate(
        [res.results[c]["out"].reshape(plan.gpc, O) for c in range(n_cores)],
        axis=0,
    )
    return out.astype(np.float32)


if __name__ == "__main__":
    # tiny smoke test of the planner only
    rng = np.random.default_rng(0)
    N, G = 400_000, 16384
    batch = np.sort(rng.integers(0, G, N))
    x = rng.standard_normal((N, D), dtype=np.float32)
    plan = Plan(batch, 8, G)
    print("lslots", plan.lslots, "nchunk", plan.nchunk, "osps", plan.osps)
    t = plan.core_tables(0, x)
    for k, v in t.items():
        print(k, v.shape, v.dtype)
